# revision 1
# baseline (speedup 1.0000x reference)
"""BasedAttention Trainium2 kernel — nn_BasedAttention_82214263980185.

Head-sharded across 8 NeuronCores (2 heads/core): column-parallel QKV,
per-head taylor linear attention (factorized phi) + banded sliding-window
attention, row-parallel out-proj with host-side partial reduction.

Math notes:
  - reference phi(x) = [1, x, tri-scaled quad] gives
    phi(q).phi(k) = 1 + s + 0.25 s^2  (s = qf.kf).  We use the equivalent
    full-outer 256-feature quad block scaled 2^-0.25 per side plus
    [x, ones]: identical inner products, rectangular construction.
  - Intra-chunk scores: A = (1 + 0.5 s)^2 = 1 + s + 0.25 s^2 directly.
  - rmsnorm: norm_w folds into QKV weights on host; the per-row 1/rms
    factor r applies to q, k, v after projection (all linear in r).
"""

import math
import sys

for _p in ("/opt/trn_rl_repo",):
    if _p not in sys.path:
        sys.path.insert(0, _p)

import numpy as np
import ml_dtypes

import concourse.bass as bass
import concourse.mybir as mybir
import concourse.tile as tile
from concourse.bass_utils import run_bass_kernel_spmd

F32 = mybir.dt.float32
BF16 = mybir.dt.bfloat16
AF = mybir.ActivationFunctionType
ALU = mybir.AluOpType
BF = ml_dtypes.bfloat16

B, T, D = 2, 2048, 1024
P = B * T          # 4096 positions
NH, DH, FT = 16, 64, 16
HPC = 2            # heads per core
NCORES = 8
WINDOW = 64
EPS_NORM = 1e-6
EPS_LIN = 1e-6
SUB = 128          # position sub-chunk (partition tile)
NSUB = P // SUB    # 32
SC = 256           # linear-attention scan chunk
NSC_B = T // SC    # 8 scan chunks per (b,h) sequence
QK_SCALE = 1.0 / math.sqrt(DH)
QUAD_PRE = 2.0 ** (-0.5)


def _fix_tile_drain():
    """walrus here accepts only 1 sync-wait on the Tile tail drain; spread
    the global-clock waits over sequencer nop carriers."""
    if getattr(tile.TileContext, "_drain_fix", False):
        return
    from concourse.tile import ScopedClock

    def _patched(self, tick_clock, wait_clock):
        nc = self.nc
        carriers = [nc.sync.nop(nofuse=True) for _ in range(30)]
        drain_inst = nc.sync.drain()
        wait_clock.add_sem_waits(
            drain_inst.ins, ScopedClock({None: tick_clock.global_clock})
        )
        si = drain_inst.ins.sync_info
        waits = list(si.on_wait) if si is not None else []
        if len(waits) > 1:
            keep, rest = waits[:1], waits[1:]
            assert len(rest) <= len(carriers), f"too many waits: {len(waits)}"
            for c, w in zip(carriers, rest):
                c.ins.sync_info = mybir.SyncInfo(on_wait=[w], on_update=[])
            drain_inst.ins.sync_info = mybir.SyncInfo(
                on_wait=keep, on_update=list(si.on_update)
            )
        nc.all_engine_barrier()
        assert self.sems is not None
        popped = nc._tile_sem_poison_stack.pop()
        assert popped is self._sem_poison
        nc.clear_and_free_semaphores(list(self.sems.allocated().values()))
        nc.all_engine_barrier()

    tile.TileContext._drain_and_barrier = _patched
    tile.TileContext._drain_fix = True


def _split_excess_waits(nc, limit=1):
    """walrus in this container rejects instructions with more than one
    embedded sync-wait; hoist excess waits onto preceding same-engine nops."""
    n = 0
    for f in nc.m.functions:
        for b in f.blocks:
            insts = b.instructions
            out = []
            changed = False
            for ins in insts:
                si = ins.sync_info
                waits = list(si.on_wait) if si is not None else []
                if len(waits) > limit:
                    changed = True
                    for w in waits[:-limit]:
                        n += 1
                        out.append(mybir.InstNoOp(
                            name=f"waitnop-{n}", engine=ins.engine,
                            bass_nofuse=True,
                            sync_info=mybir.SyncInfo(on_wait=[w],
                                                     on_update=[])))
                    ins.sync_info = mybir.SyncInfo(
                        on_wait=waits[-limit:], on_update=list(si.on_update))
                out.append(ins)
            if changed:
                b.instructions = out
    return n


def build_bass():
    _fix_tile_drain()
    nc = bass.Bass()
    dram = {}
    for name, shape in [
        ("xT", [D, P]), ("wq", [D, 128]), ("wk", [D, 128]), ("wv", [D, 128]),
        ("wfq", [128, 49]), ("wfk", [128, 49]),
        ("wqf1", [128, FT]), ("wkf1", [128, FT]),
        ("w1", [128, D]), ("w2", [128, D]),
        ("mtri", [128, 128]), ("mwd", [128, 128]), ("mwp", [128, 128]),
        ("onesP", [1, P]),
        ("ident", [128, 128]),
    ]:
        dram[name] = nc.dram_tensor(name, shape, BF16, kind="ExternalInput")
    dram["out"] = nc.dram_tensor("out", [P, D], BF16, kind="ExternalOutput")
    dram["scr1"] = nc.dram_tensor("scr1", [P], F32)
    dram["scr2"] = nc.dram_tensor("scr2", [P], F32)
    with tile.TileContext(nc) as tc:
        _emit(nc, tc, dram)
    _split_excess_waits(nc)
    return nc


def _emit(nc, tc, dram):
    from contextlib import ExitStack

    with ExitStack() as ctx:
        const = ctx.enter_context(tc.tile_pool(name="const", bufs=1))
        big = ctx.enter_context(tc.tile_pool(name="big", bufs=1))
        work = ctx.enter_context(tc.tile_pool(name="work", bufs=4))

        # ---- constants -----------------------------------------------
        cs = {}
        for name in ("ident", "mtri", "mwd", "mwp",
                     "wfq", "wfk", "wqf1", "wkf1", "w1", "w2"):
            d = dram[name]
            t_ = const.tile(list(d.shape), BF16, tag=name)
            nc.sync.dma_start(t_[:], d[:])
            cs[name] = t_
        for name in ("wq", "wk", "wv"):
            d = dram[name]
            t_ = const.tile([128, 8 * 128], BF16, tag=name)
            for kk in range(8):
                nc.sync.dma_start(t_[:, kk * 128:(kk + 1) * 128],
                                  d[kk * 128:(kk + 1) * 128, :])
            cs[name] = t_
        ones_col_b = const.tile([128, 1], BF16, tag="ocb")
        nc.gpsimd.memset(ones_col_b[:], 1.0)
        ones64_f = const.tile([1, 64], F32, tag="o64")
        nc.gpsimd.memset(ones64_f[:], 1.0)
        ones128_f = const.tile([1, 128], F32, tag="o128")
        nc.gpsimd.memset(ones128_f[:], 1.0)
        epsn_col = const.tile([128, 1], F32, tag="epsn")
        nc.gpsimd.memset(epsn_col[:], EPS_NORM)

        # ---- big persistent tiles ------------------------------------
        qT = big.tile([128, P], BF16, tag="qT")
        kT = big.tile([128, P], BF16, tag="kT")
        Vt = big.tile([128, NSUB * 130], BF16, tag="Vt")
        qfT = big.tile([64, P], BF16, tag="qfT")   # rows 16, 48 = ones
        kfT = big.tile([64, P], BF16, tag="kfT")
        catL = big.tile([128, P], BF16, tag="catL")
        catW = big.tile([128, P], BF16, tag="catW")
        r32 = big.tile([128, NSUB], F32, tag="r32")
        r_row = big.tile([1, P], F32, tag="rrow")
        sq_row = big.tile([1, P], F32, tag="sqrow")

        def vsl(gsub, h):
            base = gsub * 130 + 65 * h
            return Vt[:, base:base + 65]

        with tc.tile_pool(name="xp", bufs=1) as xp:
            xt_sb = xp.tile([128, 8 * P], BF16, tag="xt")
            xv = [xt_sb[:, kk * P:(kk + 1) * P] for kk in range(8)]
            for qq in range(4):
                csl = slice(qq * (P // 4), (qq + 1) * (P // 4))
                for kk in range(8):
                    nc.sync.dma_start(xv[kk][:, csl],
                                      dram["xT"][kk * 128:(kk + 1) * 128,
                                                 csl])

            # ---- rmsnorm scale r -------------------------------------
            with tc.tile_pool(name="psq", bufs=1, space="PSUM") as psq:
                for pc in range(8):
                    sl = slice(pc * 512, (pc + 1) * 512)
                    sq_ps = psq.tile([1, 512], F32, tag="sqps")
                    for kk in range(8):
                        sqt = work.tile([128, 512], BF16, tag="sq")
                        src = xv[kk][:, sl]
                        if kk % 2 == 0:
                            nc.scalar.activation(sqt[:], src, AF.Square)
                        else:
                            nc.vector.tensor_tensor(sqt[:], src, src, ALU.mult)
                        nc.tensor.matmul(sq_ps[:], ones_col_b[:], sqt[:],
                                         start=(kk == 0), stop=(kk == 7))
                    nc.scalar.copy(sq_row[0:1, sl], sq_ps[:])
            # (1,P) -> (128,32) via DRAM bounce: r32[o, s] = row[s*128+o]
            nc.sync.dma_start(dram["scr1"][:], sq_row[:])
            nc.sync.dma_start(
                r32[:], dram["scr1"][:].rearrange("(s o) -> o s", o=128))
            nc.scalar.activation(r32[:], r32[:], AF.Sqrt,
                                 bias=epsn_col[:], scale=1.0 / D)
            nc.vector.reciprocal(r32[:], r32[:])
            nc.sync.dma_start(
                dram["scr2"][:].rearrange("(s o) -> o s", o=128), r32[:])
            nc.sync.dma_start(r_row[:], dram["scr2"][:])

            # ---- q/k projections (d-part) ----------------------------
            wq8 = [cs["wq"][:, kk * 128:(kk + 1) * 128] for kk in range(8)]
            wk8 = [cs["wk"][:, kk * 128:(kk + 1) * 128] for kk in range(8)]
            wv8 = [cs["wv"][:, kk * 128:(kk + 1) * 128] for kk in range(8)]
            with tc.tile_pool(name="ppj", bufs=3, space="PSUM") as ppj:
                for pc in range(8):
                    sl = slice(pc * 512, (pc + 1) * 512)
                    rb_ps = ppj.tile([128, 512], F32, tag="rb")
                    nc.tensor.matmul(rb_ps[:], ones128_f[:], r_row[0:1, sl],
                                     start=True, stop=True)
                    rb_sb = work.tile([128, 512], F32, tag="rbsb")
                    nc.scalar.copy(rb_sb[:], rb_ps[:])
                    for dst, w8 in ((qT, wq8), (kT, wk8)):
                        pj = ppj.tile([128, 512], F32, tag="pj")
                        for kk in range(8):
                            nc.tensor.matmul(pj[:], w8[kk], xv[kk][:, sl],
                                             start=(kk == 0), stop=(kk == 7))
                        nc.vector.tensor_tensor(dst[:, sl], pj[:], rb_sb[:],
                                                ALU.mult)


            # ---- V (pos-part, r-scaled, ones col) --------------------
            with tc.tile_pool(name="pv", bufs=4, space="PSUM") as pv:
                for s in range(NSUB):
                    sl = slice(s * SUB, (s + 1) * SUB)
                    vp = pv.tile([128, 128], F32, tag="vp")
                    for kk in range(8):
                        nc.tensor.matmul(vp[:], xv[kk][:, sl], wv8[kk],
                                         start=(kk == 0), stop=(kk == 7))
                    rcol = r32[:, s:s + 1]
                    for h in range(HPC):
                        va = vsl(s, h)
                        nc.vector.tensor_scalar_mul(
                            va[:, 0:64], vp[:, 64 * h:64 * h + 64], rcol)
                        nc.gpsimd.memset(va[:, 64:65], 1.0)

        # ---- qfT / kfT (17-part per head, rows 16/33 ones) -----------
        with tc.tile_pool(name="pf", bufs=3, space="PSUM") as pf:
            for pc in range(8):
                sl = slice(pc * 512, (pc + 1) * 512)
                for dst, wf, src in ((qfT, cs["wfq"], qT), (kfT, cs["wfk"], kT)):
                    fp = pf.tile([49, 512], F32, tag="fp")
                    nc.tensor.matmul(fp[:], wf[:], src[:, sl],
                                     start=True, stop=True)
                    nc.vector.tensor_copy(dst[0:49, sl], fp[:])
        nc.sync.dma_start(qfT[16:17, :], dram["onesP"][:])
        nc.sync.dma_start(qfT[48:49, :], dram["onesP"][:])

        # ---- linear attention scan -----------------------------------
        with tc.tile_pool(name="pkv", bufs=1, space="PSUM") as pkv, \
             tc.tile_pool(name="psc", bufs=1, space="PSUM") as psc, \
             tc.tile_pool(name="psp", bufs=2, space="PSUM") as psp2, \
             tc.tile_pool(name="pyt", bufs=2, space="PSUM") as pyt, \
             tc.tile_pool(name="phi", bufs=17) as phip:
            for b in range(B):
                for h in range(HPC):
                    hd = slice(h * DH, (h + 1) * DH)
                    h17 = slice(h * 32, h * 32 + 17)
                    h16 = slice(h * 32, h * 32 + 16)
                    kvq = pkv.tile([128, 130], F32, tag="kvq")
                    kvlo = pkv.tile([17, 65], F32, tag="kvlo")
                    kvq_sb = work.tile([128, 130], BF16, tag="kvqs")
                    kvlo_sb = work.tile([49, 65], BF16, tag="kvlos")
                    all_q, all_k = [], []
                    for sc in range(NSC_B):
                        p0 = b * T + sc * SC
                        quads_q, quads_k = [], []
                        all_q.append(quads_q)
                        all_k.append(quads_k)
                        for cb in range(2):
                            sl = slice(p0 + cb * 128, p0 + (cb + 1) * 128)
                            qk_ps = psc.tile([128, 32], F32, tag="qkps")
                            nc.tensor.matmul(qk_ps[:, 0:16], qT[hd, sl],
                                             cs["wqf1"][hd, :], start=True,
                                             stop=True)
                            nc.tensor.matmul(qk_ps[:, 16:32], kT[hd, sl],
                                             cs["wkf1"][hd, :], start=True,
                                             stop=True)
                            qfp = phip.tile([128, FT], BF16, tag="qfp")
                            klin = phip.tile([128, 17], BF16, tag="klin")
                            # host folds 2^+0.5 into wfq, 2^-0.5 into wfk:
                            # klin doubles as the quad-scaled kf.
                            nc.scalar.activation(qfp[:], qk_ps[:, 0:16],
                                                 AF.Copy, bias=0.0,
                                                 scale=0.5)
                            nc.scalar.copy(klin[:, 0:16], qk_ps[:, 16:32])
                            nc.gpsimd.memset(klin[:, 16:17], 1.0)
                            quad_q = phip.tile([128, 256], BF16, tag="qq")
                            quad_k = phip.tile([128, 256], BF16, tag="qk")
                            for qd, fsrc in ((quad_q, qfp[:]),
                                             (quad_k, klin[:, 0:16])):
                                g1 = fsrc.unsqueeze(2).broadcast_to(
                                    (128, FT, FT))
                                g2 = fsrc.unsqueeze(1).broadcast_to(
                                    (128, FT, FT))
                                nc.gpsimd.tensor_tensor(
                                    qd[:].rearrange("p (i j) -> p i j", i=FT),
                                    g1, g2, ALU.mult)
                            q1sb = phip.tile([128, 128], BF16, tag="q1sb")
                            q2sb = phip.tile([128, 128], BF16, tag="q2sb")
                            for half, qsb in ((0, q1sb), (1, q2sb)):
                                nc.sync.dma_start_transpose(
                                    qsb[:],
                                    quad_q[:, half * 128:(half + 1) * 128])
                            quads_q.append((q1sb, q2sb))
                            quads_k.append((quad_k, klin))

                    for sc in range(NSC_B):
                        p0 = b * T + sc * SC
                        gs0 = p0 // SUB
                        quads_q = all_q[sc]
                        quads_k = all_k[sc]
                        yts = []
                        for cb in range(2):
                            sl = slice(p0 + cb * 128, p0 + (cb + 1) * 128)
                            yt = pyt.tile([65, 128], F32, tag="yt")
                            ops = []
                            for sb in range(cb + 1):
                                ssl = slice(p0 + sb * 128,
                                            p0 + (sb + 1) * 128)
                                s_ps = psp2.tile([128, 128], F32, tag="sps")
                                nc.tensor.matmul(s_ps[:], kfT[h16, ssl],
                                                 qfT[h16, sl],
                                                 start=True, stop=True)
                                a_sb = work.tile([128, 128], BF16, tag="asb")
                                nc.scalar.activation(a_sb[:], s_ps[:],
                                                     AF.Square,
                                                     bias=1.0, scale=0.5)
                                if sb == cb:
                                    nc.vector.tensor_tensor(
                                        a_sb[:], a_sb[:], cs["mtri"][:],
                                        ALU.mult)
                                ops.append((vsl(gs0 + sb, h), a_sb[:]))
                            if sc > 0:
                                q1sb, q2sb = quads_q[cb]
                                ops.append((kvq_sb[:, 0:65], q1sb[:]))
                                ops.append((kvq_sb[:, 65:130], q2sb[:]))
                                ops.append((kvlo_sb[h17, :], qfT[h17, sl]))
                            for i, (lt, rt) in enumerate(ops):
                                nc.tensor.matmul(yt[:], lt, rt,
                                                 start=(i == 0),
                                                 stop=(i == len(ops) - 1))
                            yts.append(yt)

                        for cb in range(2):
                            va = vsl(gs0 + cb, h)
                            quad_k, klin = quads_k[cb]
                            st = (sc == 0 and cb == 0)
                            sp = (sc == NSC_B - 1 and cb == 1)
                            nc.tensor.matmul(kvq[:, 0:65], quad_k[:, 0:128],
                                             va, start=st, stop=sp)
                            nc.tensor.matmul(kvq[:, 65:130],
                                             quad_k[:, 128:256], va,
                                             start=st, stop=sp)
                            nc.tensor.matmul(kvlo[:], klin[:], va,
                                             start=st, stop=sp)
                        if sc < NSC_B - 1:
                            nc.vector.tensor_copy(kvq_sb[:], kvq[:])
                            nc.vector.tensor_copy(kvlo_sb[h17, :], kvlo[:])

                        sl2 = slice(p0, p0 + SC)
                        zi = work.tile([1, 256], F32, tag="zi")
                        nc.vector.reciprocal(zi[0:1, 0:128],
                                             yts[0][64:65, :])
                        nc.vector.reciprocal(zi[0:1, 128:256],
                                             yts[1][64:65, :])
                        zb = psc.tile([64, 256], F32, tag="zb")
                        nc.tensor.matmul(zb[:], ones64_f[:], zi[:],
                                         start=True, stop=True)
                        ysb = work.tile([64, 256], BF16, tag="ysb")
                        nc.scalar.copy(ysb[:, 0:128], yts[0][0:64, :])
                        nc.scalar.copy(ysb[:, 128:256], yts[1][0:64, :])
                        nc.vector.tensor_tensor(catL[hd, sl2], ysb[:], zb[:],
                                                ALU.mult)

        # ---- sliding window attention --------------------------------
        with tc.tile_pool(name="pst", bufs=3, space="PSUM") as pst, \
             tc.tile_pool(name="pyw", bufs=3, space="PSUM") as pyw, \
             tc.tile_pool(name="pzw", bufs=2, space="PSUM") as pzw:
            for b in range(B):
                for c in range(T // SUB):
                    p0 = b * T + c * SUB
                    sl = slice(p0, p0 + SUB)
                    for h in range(HPC):
                        hd = slice(h * DH, (h + 1) * DH)
                        ytw = pyw.tile([65, 128], F32, tag="ytw")
                        sblocks = [c] if c == 0 else [c - 1, c]
                        for i, sb in enumerate(sblocks):
                            ssl = slice(b * T + sb * SUB,
                                        b * T + (sb + 1) * SUB)
                            st_ps = pst.tile([128, 128], F32, tag="stps")
                            nc.tensor.matmul(st_ps[:], kT[hd, ssl],
                                             qT[hd, sl], start=True,
                                             stop=True)
                            pexp = work.tile([128, 128], BF16, tag="pexp")
                            nc.scalar.activation(pexp[:], st_ps[:], AF.Exp,
                                                 bias=0.0, scale=QK_SCALE)
                            msk = cs["mwd"] if sb == c else cs["mwp"]
                            nc.vector.tensor_tensor(pexp[:], pexp[:], msk[:],
                                                    ALU.mult)
                            nc.tensor.matmul(
                                ytw[:], vsl(b * (T // SUB) + sb, h), pexp[:],
                                start=(i == 0),
                                stop=(i == len(sblocks) - 1))
                        ziw = work.tile([1, 128], F32, tag="ziw")
                        nc.vector.reciprocal(ziw[:], ytw[64:65, :])
                        zbw = pzw.tile([64, 128], F32, tag="zbw")
                        nc.tensor.matmul(zbw[:], ones64_f[:], ziw[:],
                                         start=True, stop=True)
                        ywsb = work.tile([64, 128], BF16, tag="ywsb")
                        nc.scalar.copy(ywsb[:], ytw[0:64, :])
                        nc.vector.tensor_tensor(catW[hd, sl], ywsb[:],
                                                zbw[:], ALU.mult)

        # ---- out-projection ------------------------------------------
        with tc.tile_pool(name="pop", bufs=3, space="PSUM") as pop, \
             tc.tile_pool(name="outp", bufs=5) as outp:
            for s in range(NSUB):
                sl = slice(s * SUB, (s + 1) * SUB)
                op = pop.tile([128, D], F32, tag="op")
                for hf in range(2):
                    c512 = slice(hf * 512, (hf + 1) * 512)
                    nc.tensor.matmul(op[:, c512], catL[:, sl],
                                     cs["w1"][:, c512], start=True, stop=False)
                    nc.tensor.matmul(op[:, c512], catW[:, sl],
                                     cs["w2"][:, c512], start=False, stop=True)
                ob = outp.tile([128, D], BF16, tag="ob")
                nc.scalar.copy(ob[:], op[:])
                nc.sync.dma_start(dram["out"][sl, :], ob[:])


_NC_CACHE = None


def _get_nc():
    global _NC_CACHE
    if _NC_CACHE is None:
        _NC_CACHE = build_bass()
    return _NC_CACHE


def _host_prep(x, norm_w, Wq, Wk, Wv, Wqf, Wkf, Wout):
    xp = np.ascontiguousarray(x.reshape(P, D).T).astype(BF)
    nw = norm_w.astype(np.float64)
    wq_f = nw[:, None] * Wq.astype(np.float64)
    wk_f = nw[:, None] * Wk.astype(np.float64)
    wv_f = nw[:, None] * Wv.astype(np.float64)

    si = np.arange(128)[:, None]
    ci = np.arange(128)[None, :]
    mtri = (si <= ci).astype(np.float32)
    mwd = ((si <= ci) & (si >= ci - WINDOW)).astype(np.float32)
    mwp = (si >= ci + WINDOW).astype(np.float32)

    sq2 = math.sqrt(2.0)
    wfq = np.zeros((128, 49), np.float32)
    wfq[0:64, 0:16] = Wqf * sq2
    wfq[64:128, 32:48] = Wqf * sq2
    wfk = np.zeros((128, 49), np.float32)
    wfk[0:64, 0:16] = Wkf / sq2
    wfk[64:128, 32:48] = Wkf / sq2

    in_maps = []
    for c in range(NCORES):
        csl = slice(c * 128, (c + 1) * 128)
        in_maps.append({
            "xT": xp,
            "wq": wq_f[:, csl].astype(BF),
            "wk": wk_f[:, csl].astype(BF),
            "wv": wv_f[:, csl].astype(BF),
            "wfq": wfq.astype(BF),
            "wfk": wfk.astype(BF),
            "wqf1": (np.vstack([Wqf, Wqf]) * sq2).astype(BF),
            "wkf1": (np.vstack([Wkf, Wkf]) / sq2).astype(BF),
            "w1": Wout[csl, :].astype(BF),
            "w2": Wout[1024 + c * 128:1024 + (c + 1) * 128, :].astype(BF),
            "mtri": mtri.astype(BF),
            "mwd": mwd.astype(BF),
            "mwp": mwp.astype(BF),
            "ident": np.eye(128, dtype=np.float32).astype(BF),
            "onesP": np.ones((1, P), np.float32).astype(BF),
        })
    return in_maps


def kernel(x, norm_w, Wq, Wk, Wv, Wqf, Wkf, Wout) -> np.ndarray:
    x = np.asarray(x, np.float32)
    in_maps = _host_prep(
        x, np.asarray(norm_w, np.float32), np.asarray(Wq, np.float32),
        np.asarray(Wk, np.float32), np.asarray(Wv, np.float32),
        np.asarray(Wqf, np.float32), np.asarray(Wkf, np.float32),
        np.asarray(Wout, np.float32))
    nc = _get_nc()
    res = run_bass_kernel_spmd(nc, in_maps, list(range(NCORES)))
    acc = np.zeros((P, D), np.float32)
    for c in range(NCORES):
        acc += res.results[c]["out"].astype(np.float32)
    return (x.reshape(P, D) + acc).reshape(B, T, D).astype(np.float32)



# revision 30
# speedup vs baseline: 1.2692x; 1.2692x over previous
"""BasedAttention Trainium2 kernel — nn_BasedAttention_82214263980185.

Head-sharded across 8 NeuronCores (2 heads/core): column-parallel QKV,
per-head taylor linear attention (factorized phi) + banded sliding-window
attention, row-parallel out-proj with host-side partial reduction.

v3: batched DMAs, quarter-granular x loads overlapped with squares,
single batched block-transpose per (b,h) for quad features, 1-col matmul
rmsnorm reduction, merged per-chunk PSUM tiles (one reciprocal / one
normalize per 256 positions), software-pipelined scan emission
(B(b,h) overlapped with A(next)) with window / out-proj filler steps.

Math notes:
  - reference phi(x) = [1, x, tri-scaled quad] gives
    phi(q).phi(k) = 1 + s + 0.25 s^2  (s = qf.kf).  We use the equivalent
    full-outer 256-feature quad block scaled 2^-0.25 per side plus
    [x, ones]: identical inner products, rectangular construction.
  - Intra-chunk scores: A = (1 + 0.5 s)^2 = 1 + s + 0.25 s^2 directly.
  - rmsnorm: norm_w folds into QKV weights on host; the per-row 1/rms
    factor r applies to q, k, v after projection (all linear in r).
"""

import math
import os
import sys

for _p in ("/opt/trn_rl_repo",):
    if _p not in sys.path:
        sys.path.insert(0, _p)

import numpy as np
import ml_dtypes

import concourse.bass as bass
import concourse.mybir as mybir
import concourse.tile as tile
from concourse.bass_utils import run_bass_kernel_spmd

F32 = mybir.dt.float32
BF16 = mybir.dt.bfloat16
AF = mybir.ActivationFunctionType
ALU = mybir.AluOpType
BF = ml_dtypes.bfloat16

B, T, D = 2, 2048, 1024
P = B * T          # 4096 positions
NH, DH, FT = 16, 64, 16
HPC = 2            # heads per core
NCORES = 8
WINDOW = 64
EPS_NORM = 1e-6
SUB = 128          # position sub-chunk (partition tile)
NSUB = P // SUB    # 32
SC = 256           # linear-attention scan chunk
NSC_B = T // SC    # 8 scan chunks per (b,h) sequence
QK_SCALE = 1.0 / math.sqrt(DH)

# weight-pack column offsets (bf16, [128, NWC])
OFF_WQ = 0
OFF_WK = 1024
OFF_WV = 2048
OFF_W1 = 3072
OFF_W2 = 4096
OFF_WFQ = 5120       # [128, 49]
OFF_WFK = 5169       # [128, 49]
OFF_WQF1 = 5218      # [128, 16]
OFF_WKF1 = 5234      # [128, 16]
OFF_MTRI2 = 5250     # [128, 256] = [tril | tril]
OFF_WINM = 5506      # [128, 512] = [mwp | mwd | mwp | mwd]
NWC = 6018


def _fix_tile_drain():
    """walrus here accepts only 1 sync-wait on the Tile tail drain; spread
    the global-clock waits over sequencer nop carriers."""
    if getattr(tile.TileContext, "_drain_fix", False):
        return
    from concourse.tile import ScopedClock

    def _patched(self, tick_clock, wait_clock):
        nc = self.nc
        carriers = [nc.sync.nop(nofuse=True) for _ in range(30)]
        drain_inst = nc.sync.drain()
        wait_clock.add_sem_waits(
            drain_inst.ins, ScopedClock({None: tick_clock.global_clock})
        )
        si = drain_inst.ins.sync_info
        waits = list(si.on_wait) if si is not None else []
        if len(waits) > 1:
            keep, rest = waits[:1], waits[1:]
            assert len(rest) <= len(carriers), f"too many waits: {len(waits)}"
            for c, w in zip(carriers, rest):
                c.ins.sync_info = mybir.SyncInfo(on_wait=[w], on_update=[])
            drain_inst.ins.sync_info = mybir.SyncInfo(
                on_wait=keep, on_update=list(si.on_update)
            )
        nc.all_engine_barrier()
        assert self.sems is not None
        popped = nc._tile_sem_poison_stack.pop()
        assert popped is self._sem_poison
        nc.clear_and_free_semaphores(list(self.sems.allocated().values()))
        nc.all_engine_barrier()

    tile.TileContext._drain_and_barrier = _patched
    tile.TileContext._drain_fix = True


def _split_excess_waits(nc, limit=1):
    """walrus in this container rejects instructions with more than one
    embedded sync-wait; hoist excess waits onto preceding same-engine nops."""
    n = 0
    for f in nc.m.functions:
        for b in f.blocks:
            insts = b.instructions
            out = []
            changed = False
            for ins in insts:
                si = ins.sync_info
                waits = list(si.on_wait) if si is not None else []
                if len(waits) > limit:
                    changed = True
                    for w in waits[:-limit]:
                        n += 1
                        out.append(mybir.InstNoOp(
                            name=f"waitnop-{n}", engine=ins.engine,
                            bass_nofuse=True,
                            sync_info=mybir.SyncInfo(on_wait=[w],
                                                     on_update=[])))
                    ins.sync_info = mybir.SyncInfo(
                        on_wait=waits[-limit:], on_update=list(si.on_update))
                out.append(ins)
            if changed:
                b.instructions = out
    return n


def build_bass():
    _fix_tile_drain()
    nc = bass.Bass()
    dram = {}
    dram["xT"] = nc.dram_tensor("xT", [D, P], BF16, kind="ExternalInput")
    dram["wpack"] = nc.dram_tensor("wpack", [128, NWC], BF16,
                                   kind="ExternalInput")
    dram["identf"] = nc.dram_tensor("identf", [128, 128], F32,
                                    kind="ExternalInput")
    dram["onesP"] = nc.dram_tensor("onesP", [1, P], BF16,
                                   kind="ExternalInput")
    dram["out"] = nc.dram_tensor("out", [P, D], BF16, kind="ExternalOutput")
    dram["scr_r"] = nc.dram_tensor("scr_r", [P], BF16)
    with tile.TileContext(nc) as tc:
        _emit(nc, tc, dram)
    _split_excess_waits(nc)
    return nc


def _interleave(streams):
    """streams: list of (generator, weight). Round-robin: advance each
    generator up to `weight` steps per round until all are exhausted."""
    live = [[g, w] for g, w in streams]
    while live:
        nxt = []
        for g, w in live:
            alive = True
            for _ in range(w):
                try:
                    next(g)
                except StopIteration:
                    alive = False
                    break
            if alive:
                nxt.append([g, w])
        live = nxt


def _take(lst, n):
    """Generator yielding up to n popped steps from lst (executing them)."""
    for _ in range(n):
        if not lst:
            return
        lst.pop(0)()
        yield


def _emit(nc, tc, dram):
    from contextlib import ExitStack

    with ExitStack() as ctx:
        const = ctx.enter_context(tc.tile_pool(name="const", bufs=1))
        big = ctx.enter_context(tc.tile_pool(name="big", bufs=1))
        work = ctx.enter_context(tc.tile_pool(name="work", bufs=4))

        # ---- constants -----------------------------------------------
        wsb = const.tile([128, NWC], BF16, tag="wsb")
        nc.sync.dma_start(wsb[:], dram["wpack"][:])
        identf = const.tile([128, 128], F32, tag="identf")
        nc.sync.dma_start(identf[:], dram["identf"][:])
        ones_col_b = const.tile([128, 1], BF16, tag="ocb")
        nc.gpsimd.memset(ones_col_b[:], 1.0)
        ones64_f = const.tile([1, 64], F32, tag="o64")
        nc.gpsimd.memset(ones64_f[:], 1.0)
        ones128_b = const.tile([1, 128], BF16, tag="o128")
        nc.gpsimd.memset(ones128_b[:], 1.0)
        epsn_col = const.tile([128, 1], F32, tag="epsn")
        nc.gpsimd.memset(epsn_col[:], EPS_NORM)

        wq8 = [wsb[:, OFF_WQ + kk * 128:OFF_WQ + (kk + 1) * 128]
               for kk in range(8)]
        wk8 = [wsb[:, OFF_WK + kk * 128:OFF_WK + (kk + 1) * 128]
               for kk in range(8)]
        wv8 = [wsb[:, OFF_WV + kk * 128:OFF_WV + (kk + 1) * 128]
               for kk in range(8)]

        # ---- big persistent tiles ------------------------------------
        qT = big.tile([128, P], BF16, tag="qT")
        kT = big.tile([128, P], BF16, tag="kT")
        Vt = big.tile([128, NSUB * 130], BF16, tag="Vt")
        qfT = big.tile([64, P], BF16, tag="qfT")   # rows 16, 48 = ones
        kfT = big.tile([64, P], BF16, tag="kfT")
        catL = big.tile([128, P], BF16, tag="catL")
        catW = big.tile([128, P], BF16, tag="catW")
        r32 = big.tile([128, NSUB], F32, tag="r32")
        r32T = big.tile([8, 512], BF16, tag="r32T")
        rb_all = big.tile([128, P], BF16, tag="rball")

        def vsl(gsub, h):
            base = gsub * 130 + 65 * h
            return Vt[:, base:base + 65]

        # ---- phase A: rmsnorm, q/k/v projections, feature maps -------
        with tc.tile_pool(name="xp", bufs=1) as xp, \
             tc.tile_pool(name="psA", bufs=1, space="PSUM") as psA:
            xt_sb = xp.tile([128, 8 * P], BF16, tag="xt")
            xv = [xt_sb[:, kk * P:(kk + 1) * P] for kk in range(8)]
            for q in range(4):
                qsl = slice(q * 1024, (q + 1) * 1024)
                for kk in range(8):
                    nc.sync.dma_start(xv[kk][:, qsl],
                                      dram["xT"][kk * 128:(kk + 1) * 128,
                                                 qsl])

            # per-quarter pipeline: squares -> r(quarter) -> q/k proj ->
            # V -> feature maps, so nothing waits on a global r barrier
            sq_ps = psA.tile([128, NSUB], F32, tag="sq")
            nc.gpsimd.memset(
                Vt[:].rearrange("p (s h o) -> p s h o", h=2, o=65)[:, :, :, 64],
                1.0)
            for q in range(4):
                qsl = slice(q * 1024, (q + 1) * 1024)
                q8 = slice(q * 8, (q + 1) * 8)
                sqts = []
                for kk in range(8):
                    sqt = work.tile([128, 1024], BF16, tag="sqt", bufs=8)
                    src = xv[kk][:, qsl]
                    if kk % 2 == 0:
                        nc.scalar.activation(sqt[:], src, AF.Square)
                    else:
                        nc.vector.tensor_tensor(sqt[:], src, src, ALU.mult)
                    sqts.append(sqt)
                for sub in range(8):
                    col = q * 8 + sub
                    for kk in range(8):
                        nc.tensor.matmul(
                            sq_ps[:, col:col + 1],
                            sqts[kk][:, sub * 128:(sub + 1) * 128],
                            ones_col_b[:], start=(kk == 0), stop=(kk == 7))
                # r for this quarter: [128, 8] column block, transposed into
                # a [1, 1024] slice of r_row via PE transpose + sbuf DMA
                nc.scalar.activation(r32[:, q8], sq_ps[:, q8], AF.Sqrt,
                                     bias=epsn_col[:], scale=1.0 / D)
                nc.vector.reciprocal(r32[:, q8], r32[:, q8])
                rT_ps = psA.tile([8, 128], F32, tag="rT")
                nc.tensor.transpose(rT_ps[:], r32[:, q8], identf[:])
                rtc = slice(q * 128, (q + 1) * 128)
                nc.scalar.copy(r32T[0:8, rtc], rT_ps[:])
                nc.sync.dma_start(
                    dram["scr_r"][qsl].rearrange("(s c) -> s c", c=128),
                    r32T[0:8, rtc])
                nc.sync.dma_start(
                    rb_all[:, qsl],
                    dram["scr_r"][qsl].unsqueeze(0).broadcast_to((128, 1024)))
                # q/k projections (scaled), then feature maps, this quarter
                for pc in (2 * q, 2 * q + 1):
                    sl = slice(pc * 512, (pc + 1) * 512)
                    for dst, w8 in ((qT, wq8), (kT, wk8)):
                        pj = psA.tile([128, 512], F32, tag="pj", bufs=2)
                        for kk in range(8):
                            nc.tensor.matmul(pj[:], w8[kk], xv[kk][:, sl],
                                             start=(kk == 0), stop=(kk == 7))
                        nc.vector.tensor_tensor(dst[:, sl], pj[:],
                                                rb_all[:, sl], ALU.mult)
                    for i, (dstT, woff, srcT) in enumerate(
                            ((qfT, OFF_WFQ, qT), (kfT, OFF_WFK, kT))):
                        fp = psA.tile([49, 512], F32, tag="rb", bufs=2)
                        nc.tensor.matmul(fp[:], wsb[:, woff:woff + 49],
                                         srcT[:, sl], start=True, stop=True)
                        if (pc + i) % 2 == 0:
                            nc.scalar.copy(dstT[0:49, sl], fp[:])
                        else:
                            nc.vector.tensor_copy(dstT[0:49, sl], fp[:])
                # V for this quarter
                for s in range(q * 8, (q + 1) * 8):
                    sl = slice(s * SUB, (s + 1) * SUB)
                    vp = psA.tile([128, 128], F32, tag="vp", bufs=2)
                    for kk in range(8):
                        nc.tensor.matmul(vp[:], xv[kk][:, sl], wv8[kk],
                                         start=(kk == 0), stop=(kk == 7))
                    base = s * 130
                    dst = Vt[:, base:base + 130].rearrange(
                        "p (h x) -> p h x", x=65)[:, :, 0:64]
                    srcv = vp[:].rearrange("p (h x) -> p h x", x=64)
                    nc.vector.tensor_scalar_mul(dst, srcv, r32[:, s:s + 1])

        nc.sync.dma_start(qfT[16:17, :], dram["onesP"][:])
        nc.sync.dma_start(qfT[48:49, :], dram["onesP"][:])

        # ---- scan + window + out-proj, software-pipelined ------------
        def make_win_step(psW, winw, b, c):
            def step():
                p0 = b * T + c * SUB
                slq = slice(p0, p0 + SUB)
                g = b * (T // SUB) + c
                nblk = 2 if c == 0 else 4
                # separate [128,128] score tiles (one matmul group per psum
                # bank); exp lands in slices of one staging tile so the mask
                # multiply stays batched
                pexp = winw.tile([128, 512], BF16, tag="pexp")
                for h in range(HPC):
                    hd = slice(h * DH, (h + 1) * DH)
                    sbs = (c,) if c == 0 else (c - 1, c)
                    for i, sb in enumerate(sbs):
                        ssl = slice(b * T + sb * SUB, b * T + (sb + 1) * SUB)
                        stt = psW.tile([128, 128], F32, tag="st", bufs=2,
                                       name="stt")
                        nc.tensor.matmul(stt[:], kT[hd, ssl], qT[hd, slq],
                                         start=True, stop=True)
                        blk = (len(sbs) * h + i) * 128
                        nc.scalar.activation(pexp[:, blk:blk + 128], stt[:],
                                             AF.Exp, bias=0.0,
                                             scale=QK_SCALE)
                pexp2 = winw.tile([128, 512], BF16, tag="pexp2")
                if c == 0:
                    mview = wsb[:, OFF_WINM:OFF_WINM + 512].rearrange(
                        "p (a x) -> p a x", x=256)[:, :, 128:256]
                    nc.gpsimd.tensor_tensor(
                        pexp2[:, 0:256].rearrange("p (a x) -> p a x", x=128),
                        pexp[:, 0:256].rearrange("p (a x) -> p a x", x=128),
                        mview, ALU.mult)
                else:
                    nc.vector.tensor_tensor(
                        pexp2[:, 0:256], pexp[:, 0:256],
                        wsb[:, OFF_WINM:OFF_WINM + 256], ALU.mult)
                    nc.gpsimd.tensor_tensor(
                        pexp2[:, 256:512], pexp[:, 256:512],
                        wsb[:, OFF_WINM + 256:OFF_WINM + 512], ALU.mult)
                ytw2 = psW.tile([65, 256], F32, tag="yw", bufs=1)
                for h in range(HPC):
                    if c == 0:
                        nc.tensor.matmul(
                            ytw2[:, h * 128:(h + 1) * 128], vsl(g, h),
                            pexp2[:, h * 128:(h + 1) * 128],
                            start=True, stop=True)
                    else:
                        nc.tensor.matmul(ytw2[:, h * 128:(h + 1) * 128],
                                         vsl(g - 1, h),
                                         pexp2[:, (2 * h) * 128:
                                               (2 * h) * 128 + 128],
                                         start=True, stop=False)
                        nc.tensor.matmul(ytw2[:, h * 128:(h + 1) * 128],
                                         vsl(g, h),
                                         pexp2[:, (2 * h + 1) * 128:
                                               (2 * h + 1) * 128 + 128],
                                         start=False, stop=True)
                ziw = winw.tile([1, 256], F32, tag="ziw", bufs=2)
                nc.vector.reciprocal(ziw[:], ytw2[64:65, :])
                zbw2 = psW.tile([128, 128], F32, tag="yw", bufs=1)
                nc.tensor.matmul(zbw2[0:64, :], ones64_f[:],
                                 ziw[0:1, 0:128], start=True, stop=True)
                nc.tensor.matmul(zbw2[64:128, :], ones64_f[:],
                                 ziw[0:1, 128:256], start=True, stop=True)
                ywsb2 = winw.tile([128, 128], BF16, tag="ywsb", bufs=2)
                nc.scalar.copy(ywsb2[0:64, :], ytw2[0:64, 0:128])
                nc.scalar.copy(ywsb2[64:128, :], ytw2[0:64, 128:256])
                nc.vector.tensor_tensor(catW[:, slq], ywsb2[:], zbw2[:],
                                        ALU.mult)
            return step

        out_shared = {}

        def make_out_step(psO, outw, b, c):
            def step():
                s = b * 16 + c
                sl = slice(s * SUB, (s + 1) * SUB)
                if c % 4 == 0:
                    ob4 = outw.tile([128, 4 * D], BF16, tag="obuf",
                                    bufs=2, name="ob4")
                    out_shared[b] = ob4
                obuf = out_shared[b]
                for hf in range(2):
                    op = psO.tile([128, 512], F32, tag="op", bufs=3)
                    nc.tensor.matmul(op[:], catL[:, sl],
                                     wsb[:, OFF_W1 + hf * 512:
                                         OFF_W1 + (hf + 1) * 512],
                                     start=True, stop=False)
                    nc.tensor.matmul(op[:], catW[:, sl],
                                     wsb[:, OFF_W2 + hf * 512:
                                         OFF_W2 + (hf + 1) * 512],
                                     start=False, stop=True)
                    dsl = slice((c % 4) * D + hf * 512,
                                (c % 4) * D + (hf + 1) * 512)
                    if hf == 0:
                        nc.scalar.copy(obuf[:, dsl], op[:])
                    else:
                        nc.vector.tensor_copy(obuf[:, dsl], op[:])
                if c % 4 == 3:
                    s0 = (s - 3) * SUB
                    dst = dram["out"][s0:s0 + 512, :].rearrange(
                        "(t p) d -> p t d", p=128)
                    nc.sync.dma_start(dst, obuf[:])
            return step

        scan_shared = {}

        def gen_A(b, h, psS, scanw):
            hd = slice(h * DH, (h + 1) * DH)
            hd2 = slice(h * 64, h * 64 + 64)
            qq_all = scanw.tile([128, 16 * 256], BF16, tag="qqall", bufs=2)
            qqT_all = scanw.tile([128, 16 * 256], BF16, tag="qqTall", bufs=2)
            qk_all = scanw.tile([128, 16 * 256], BF16, tag="qkall", bufs=2)
            feat_all = scanw.tile([128, 16 * 34], BF16, tag="ftall",
                                  bufs=2)
            scan_shared[(b, h)] = (qq_all, qqT_all, qk_all, feat_all)
            nc.gpsimd.memset(
                feat_all[:].rearrange("p (t c) -> p t c", c=34)[:, :, 32],
                1.0)
            # all qf/kf projections of this (b,h) into one PSUM tile,
            # then a single strided copy into the feature slab
            qk_ps = psS.tile([128, 512], F32, tag="scr", bufs=2)
            for t in range(16):
                p0 = b * T + t * 128
                sl = slice(p0, p0 + 128)
                nc.tensor.matmul(qk_ps[:, t * 32:t * 32 + 16], qT[hd, sl],
                                 wsb[hd2, OFF_WQF1:OFF_WQF1 + 16],
                                 start=True, stop=True)
                nc.tensor.matmul(qk_ps[:, t * 32 + 16:t * 32 + 32],
                                 kT[hd, sl],
                                 wsb[hd2, OFF_WKF1:OFF_WKF1 + 16],
                                 start=True, stop=True)
            nc.scalar.copy(
                feat_all[:].rearrange("p (t c) -> p t c", c=34)[:, :, 0:32],
                qk_ps[:].rearrange("p (t c) -> p t c", c=32))
            yield
            for sc in range(NSC_B):
                for cb in range(2):
                    t = 2 * sc + cb
                    ft = feat_all[:, t * 34:t * 34 + 34]
                    qv = qq_all[:, t * 256:(t + 1) * 256].rearrange(
                        "p (i j) -> p i j", i=16)
                    kv_ = qk_all[:, t * 256:(t + 1) * 256].rearrange(
                        "p (i j) -> p i j", i=16)
                    qg1 = ft[:, 0:16].unsqueeze(2).broadcast_to(
                        (128, FT, FT))
                    qg2 = ft[:, 0:16].unsqueeze(1).broadcast_to(
                        (128, FT, FT))
                    kg1 = ft[:, 16:32].unsqueeze(2).broadcast_to(
                        (128, FT, FT))
                    kg2 = ft[:, 16:32].unsqueeze(1).broadcast_to(
                        (128, FT, FT))
                    nc.vector.tensor_tensor(qv[:, 0:8, :], qg1[:, 0:8, :],
                                            qg2[:, 0:8, :], ALU.mult)
                    nc.gpsimd.tensor_tensor(qv[:, 8:16, :], qg1[:, 8:16, :],
                                            qg2[:, 8:16, :], ALU.mult)
                    nc.gpsimd.tensor_tensor(kv_[:, 0:8, :], kg1[:, 0:8, :],
                                            kg2[:, 0:8, :], ALU.mult)
                    nc.vector.tensor_tensor(kv_[:, 8:16, :], kg1[:, 8:16, :],
                                            kg2[:, 8:16, :], ALU.mult)
                    yield
            nc.sync.dma_start_transpose(
                qqT_all[:].rearrange("p (t c) -> p t c", c=128), qq_all[:])

        def gen_B(b, h, psS, scanw):
            hd = slice(h * DH, (h + 1) * DH)
            h16 = slice(h * 32, h * 32 + 16)
            h17 = slice(h * 32, h * 32 + 17)
            _, qqT_all, qk_all, feat_all = scan_shared.pop((b, h))
            kv_sb = scanw.tile([128, 196], BF16, tag="kvsb", bufs=2)
            lo0 = 0 if h == 0 else 32
            for sc in range(NSC_B):
                p0 = b * T + sc * SC
                gs0 = p0 // SUB
                sl0 = slice(p0, p0 + 128)
                sl1 = slice(p0 + 128, p0 + 256)
                # intra-chunk score trio: [diag0 | diag1 | off(0->1)]
                strio = psS.tile([128, 384], F32, tag="scr", bufs=2)
                nc.tensor.matmul(strio[:, 0:128], kfT[h16, sl0],
                                 qfT[h16, sl0], start=True, stop=True)
                nc.tensor.matmul(strio[:, 128:256], kfT[h16, sl1],
                                 qfT[h16, sl1], start=True, stop=True)
                nc.tensor.matmul(strio[:, 256:384], kfT[h16, sl0],
                                 qfT[h16, sl1], start=True, stop=True)
                a_all = work.tile([128, 384], BF16, tag="asb", bufs=2)
                nc.scalar.activation(a_all[:], strio[:], AF.Square,
                                     bias=1.0, scale=0.5)
                nc.vector.tensor_tensor(
                    a_all[:, 0:256], a_all[:, 0:256],
                    wsb[:, OFF_MTRI2:OFF_MTRI2 + 256], ALU.mult)

                yt = psS.tile([65, 256], F32, tag="yt", bufs=2)
                for cb in range(2):
                    sl = sl0 if cb == 0 else sl1
                    t = 2 * sc + cb
                    yv = yt[:, cb * 128:(cb + 1) * 128]
                    ops = []
                    if cb == 0:
                        ops.append((vsl(gs0, h), a_all[:, 0:128]))
                    else:
                        ops.append((vsl(gs0, h), a_all[:, 256:384]))
                        ops.append((vsl(gs0 + 1, h), a_all[:, 128:256]))
                    if sc > 0:
                        ops.append((kv_sb[:, 0:65],
                                    qqT_all[:, (2 * t) * 128:
                                            (2 * t) * 128 + 128]))
                        ops.append((kv_sb[:, 65:130],
                                    qqT_all[:, (2 * t + 1) * 128:
                                            (2 * t + 1) * 128 + 128]))
                        ops.append((kv_sb[h17, 130:195], qfT[h17, sl]))
                    for i, (lt, rt) in enumerate(ops):
                        nc.tensor.matmul(yv, lt, rt, start=(i == 0),
                                         stop=(i == len(ops) - 1))

                # state update: per-sc delta (short psum groups), then
                # bf16 state accumulate in SBUF
                if sc < NSC_B - 1:
                    kv = psS.tile([128, 196], F32, tag="kv", bufs=1)
                    t0, t1 = 2 * sc, 2 * sc + 1
                    va0, va1 = vsl(gs0, h), vsl(gs0 + 1, h)
                    for lo, hi, src0, src1 in (
                        (0, 65, qk_all[:, t0 * 256:t0 * 256 + 128],
                         qk_all[:, t1 * 256:t1 * 256 + 128]),
                        (65, 130, qk_all[:, t0 * 256 + 128:(t0 + 1) * 256],
                         qk_all[:, t1 * 256 + 128:(t1 + 1) * 256]),
                    ):
                        nc.tensor.matmul(kv[:, lo:hi], src0, va0,
                                         start=True, stop=False)
                        nc.tensor.matmul(kv[:, lo:hi], src1, va1,
                                         start=False, stop=True)
                    nc.tensor.matmul(kv[lo0:lo0 + 17, 130:195],
                                     feat_all[:, t0 * 34 + 16:t0 * 34 + 33],
                                     va0, start=True, stop=False)
                    nc.tensor.matmul(kv[lo0:lo0 + 17, 130:195],
                                     feat_all[:, t1 * 34 + 16:t1 * 34 + 33],
                                     va1, start=False, stop=True)
                    kq = kv[:, 0:130]
                    klo = kv[lo0:lo0 + 17, 130:195]
                    if sc == 0:
                        nc.vector.tensor_copy(kv_sb[:, 0:130], kq)
                        nc.vector.tensor_copy(kv_sb[h17, 130:195], klo)
                    else:
                        nc.vector.tensor_tensor(kv_sb[:, 0:130],
                                                kv_sb[:, 0:130], kq,
                                                ALU.add)
                        nc.vector.tensor_tensor(kv_sb[h17, 130:195],
                                                kv_sb[h17, 130:195], klo,
                                                ALU.add)

                # normalize into catL
                zi = work.tile([1, 256], F32, tag="zi", bufs=2)
                nc.vector.reciprocal(zi[:], yt[64:65, :])
                zb = psS.tile([64, 256], F32, tag="scr", bufs=2)
                nc.tensor.matmul(zb[:], ones64_f[:], zi[:],
                                 start=True, stop=True)
                zb_sb = work.tile([64, 256], BF16, tag="zbsb", bufs=2)
                nc.scalar.copy(zb_sb[:], zb[:])
                nc.vector.tensor_tensor(
                    catL[hd, slice(p0, p0 + 256)], yt[0:64, :], zb_sb[:],
                    ALU.mult)
                yield

        # drive the pipelined emission. PSUM budget: psS (kv 1 + scr 2
        # + yt 2 = 5 banks) spans everything; psW (st 1 + yw 2 = 3) lives
        # through stage 2 (all windows); psO (op 3) for stages 3-5.
        with tc.tile_pool(name="scanw", bufs=2) as scanw, \
             tc.tile_pool(name="psS", bufs=2, space="PSUM") as psS:
            with tc.tile_pool(name="winw", bufs=2) as winw, \
                 tc.tile_pool(name="psW", bufs=1, space="PSUM") as psW:
                win_steps = [make_win_step(psW, winw, b, c)
                             for b in range(B) for c in range(T // SUB)]
                # stage 0: A(0,0) + 10 win
                _interleave([(gen_A(0, 0, psS, scanw), 2),
                             (_take(win_steps, 10), 1)])
                # stage 1: B(0,0) + A(0,1) + 11 win
                _interleave([(gen_B(0, 0, psS, scanw), 1),
                             (gen_A(0, 1, psS, scanw), 2),
                             (_take(win_steps, 11), 1)])
                # stage 2: B(0,1) + A(1,0) + rest of win
                _interleave([(gen_B(0, 1, psS, scanw), 1),
                             (gen_A(1, 0, psS, scanw), 2),
                             (_take(win_steps, 11), 1)])
                for stp in win_steps:
                    stp()
                win_steps.clear()
            with tc.tile_pool(name="outw", bufs=2) as outw, \
                 tc.tile_pool(name="psO", bufs=3, space="PSUM") as psO:
                out_b0 = [make_out_step(psO, outw, 0, c) for c in range(16)]
                out_b1 = [make_out_step(psO, outw, 1, c) for c in range(16)]
                # stage 3: B(1,0) + A(1,1) + 8 out(b0)
                _interleave([(gen_B(1, 0, psS, scanw), 1),
                             (gen_A(1, 1, psS, scanw), 2),
                             (_take(out_b0, 8), 1)])
                # stage 4: B(1,1) + rest of out(b0) + out(b1) as catL(b1)
                # columns land (out(b1,c) needs B(1,1) sc >= c//2)
                tail = []
                for i in range(8):
                    if out_b0:
                        tail.append(out_b0.pop(0))
                    if out_b1:
                        tail.append(out_b1[2 * i])
                        tail.append(out_b1[2 * i + 1])
                _interleave([(gen_B(1, 1, psS, scanw), 1),
                             (_take(tail, len(tail)), 3)])
                for stp in tail:
                    stp()


_NC_CACHE = None
def _get_nc():
    global _NC_CACHE
    if _NC_CACHE is None:
        _NC_CACHE = build_bass()
    return _NC_CACHE


def _host_prep(x, norm_w, Wq, Wk, Wv, Wqf, Wkf, Wout):
    xp = np.ascontiguousarray(x.reshape(P, D).T).astype(BF)
    nw = norm_w.astype(np.float64)
    wq_f = nw[:, None] * Wq.astype(np.float64)
    wk_f = nw[:, None] * Wk.astype(np.float64)
    wv_f = nw[:, None] * Wv.astype(np.float64)

    si = np.arange(128)[:, None]
    ci = np.arange(128)[None, :]
    mtri = (si <= ci).astype(np.float32)
    mwd = ((si <= ci) & (si >= ci - WINDOW)).astype(np.float32)
    mwp = (si >= ci + WINDOW).astype(np.float32)
    mtri2 = np.concatenate([mtri, mtri], 1)
    winm = np.concatenate([mwp, mwd, mwp, mwd], 1)

    sq2 = math.sqrt(2.0)
    wfq = np.zeros((128, 49), np.float32)
    wfq[0:64, 0:16] = Wqf * sq2
    wfq[64:128, 32:48] = Wqf * sq2
    wfk = np.zeros((128, 49), np.float32)
    wfk[0:64, 0:16] = Wkf / sq2
    wfk[64:128, 32:48] = Wkf / sq2
    wqf1 = np.vstack([Wqf, Wqf]) / sq2
    wkf1 = np.vstack([Wkf, Wkf]) / sq2

    in_maps = []
    for c in range(NCORES):
        csl = slice(c * 128, (c + 1) * 128)
        wq_sb = wq_f[:, csl].reshape(8, 128, 128).transpose(1, 0, 2).reshape(
            128, 1024)
        wk_sb = wk_f[:, csl].reshape(8, 128, 128).transpose(1, 0, 2).reshape(
            128, 1024)
        wv_sb = wv_f[:, csl].reshape(8, 128, 128).transpose(1, 0, 2).reshape(
            128, 1024)
        wpack = np.zeros((128, NWC), np.float32)
        wpack[:, OFF_WQ:OFF_WQ + 1024] = wq_sb
        wpack[:, OFF_WK:OFF_WK + 1024] = wk_sb
        wpack[:, OFF_WV:OFF_WV + 1024] = wv_sb
        wpack[:, OFF_W1:OFF_W1 + 1024] = Wout[csl, :]
        wpack[:, OFF_W2:OFF_W2 + 1024] = Wout[1024 + c * 128:
                                              1024 + (c + 1) * 128, :]
        wpack[:, OFF_WFQ:OFF_WFQ + 49] = wfq
        wpack[:, OFF_WFK:OFF_WFK + 49] = wfk
        wpack[:, OFF_WQF1:OFF_WQF1 + 16] = wqf1
        wpack[:, OFF_WKF1:OFF_WKF1 + 16] = wkf1
        wpack[:, OFF_MTRI2:OFF_MTRI2 + 256] = mtri2
        wpack[:, OFF_WINM:OFF_WINM + 512] = winm
        in_maps.append({
            "xT": xp,
            "wpack": wpack.astype(BF),
            "identf": np.eye(128, dtype=np.float32),
            "onesP": np.ones((1, P), np.float32).astype(BF),
        })
    return in_maps


def kernel(x, norm_w, Wq, Wk, Wv, Wqf, Wkf, Wout) -> np.ndarray:
    x = np.asarray(x, np.float32)
    in_maps = _host_prep(
        x, np.asarray(norm_w, np.float32), np.asarray(Wq, np.float32),
        np.asarray(Wk, np.float32), np.asarray(Wv, np.float32),
        np.asarray(Wqf, np.float32), np.asarray(Wkf, np.float32),
        np.asarray(Wout, np.float32))
    nc = _get_nc()
    res = run_bass_kernel_spmd(nc, in_maps, list(range(NCORES)))
    acc = np.zeros((P, D), np.float32)
    for c in range(NCORES):
        acc += res.results[c]["out"].astype(np.float32)
    return (x.reshape(P, D) + acc).reshape(B, T, D).astype(np.float32)


# revision 34
# speedup vs baseline: 1.2916x; 1.0176x over previous
"""BasedAttention Trainium2 kernel — nn_BasedAttention_82214263980185.

Head-sharded across 8 NeuronCores (2 heads/core): column-parallel QKV,
per-head taylor linear attention (factorized phi) + banded sliding-window
attention, row-parallel out-proj with host-side partial reduction.

v3: batched DMAs, quarter-granular x loads overlapped with squares,
single batched block-transpose per (b,h) for quad features, 1-col matmul
rmsnorm reduction, merged per-chunk PSUM tiles (one reciprocal / one
normalize per 256 positions), software-pipelined scan emission
(B(b,h) overlapped with A(next)) with window / out-proj filler steps.

Math notes:
  - reference phi(x) = [1, x, tri-scaled quad] gives
    phi(q).phi(k) = 1 + s + 0.25 s^2  (s = qf.kf).  We use the equivalent
    full-outer 256-feature quad block scaled 2^-0.25 per side plus
    [x, ones]: identical inner products, rectangular construction.
  - Intra-chunk scores: A = (1 + 0.5 s)^2 = 1 + s + 0.25 s^2 directly.
  - rmsnorm: norm_w folds into QKV weights on host; the per-row 1/rms
    factor r applies to q, k, v after projection (all linear in r).
"""

import math
import os
import sys

for _p in ("/opt/trn_rl_repo",):
    if _p not in sys.path:
        sys.path.insert(0, _p)

import numpy as np
import ml_dtypes

import concourse.bass as bass
import concourse.mybir as mybir
import concourse.tile as tile
from concourse.bass_utils import run_bass_kernel_spmd

F32 = mybir.dt.float32
BF16 = mybir.dt.bfloat16
AF = mybir.ActivationFunctionType
ALU = mybir.AluOpType
BF = ml_dtypes.bfloat16

B, T, D = 2, 2048, 1024
P = B * T          # 4096 positions
NH, DH, FT = 16, 64, 16
HPC = 2            # heads per core
NCORES = 8
WINDOW = 64
EPS_NORM = 1e-6
SUB = 128          # position sub-chunk (partition tile)
NSUB = P // SUB    # 32
SC = 256           # linear-attention scan chunk
NSC_B = T // SC    # 8 scan chunks per (b,h) sequence
QK_SCALE = 1.0 / math.sqrt(DH)

# weight-pack column offsets (bf16, [128, NWC])
OFF_WQ = 0
OFF_WK = 1024
OFF_WV = 2048
OFF_W1 = 3072
OFF_W2 = 4096
OFF_WFQ = 5120       # [128, 49]
OFF_WFK = 5169       # [128, 49]
OFF_WQF1 = 5218      # [128, 16]
OFF_WKF1 = 5234      # [128, 16]
OFF_MTRI2 = 5250     # [128, 256] = [tril | tril]
OFF_WINM = 5506      # [128, 512] = [mwp | mwd | mwp | mwd]
NWC = 6018


def _fix_tile_drain():
    """walrus here accepts only 1 sync-wait on the Tile tail drain; spread
    the global-clock waits over sequencer nop carriers."""
    if getattr(tile.TileContext, "_drain_fix", False):
        return
    from concourse.tile import ScopedClock

    def _patched(self, tick_clock, wait_clock):
        nc = self.nc
        carriers = [nc.sync.nop(nofuse=True) for _ in range(30)]
        drain_inst = nc.sync.drain()
        wait_clock.add_sem_waits(
            drain_inst.ins, ScopedClock({None: tick_clock.global_clock})
        )
        si = drain_inst.ins.sync_info
        waits = list(si.on_wait) if si is not None else []
        if len(waits) > 1:
            keep, rest = waits[:1], waits[1:]
            assert len(rest) <= len(carriers), f"too many waits: {len(waits)}"
            for c, w in zip(carriers, rest):
                c.ins.sync_info = mybir.SyncInfo(on_wait=[w], on_update=[])
            drain_inst.ins.sync_info = mybir.SyncInfo(
                on_wait=keep, on_update=list(si.on_update)
            )
        nc.all_engine_barrier()
        assert self.sems is not None
        popped = nc._tile_sem_poison_stack.pop()
        assert popped is self._sem_poison
        nc.clear_and_free_semaphores(list(self.sems.allocated().values()))
        nc.all_engine_barrier()

    tile.TileContext._drain_and_barrier = _patched
    tile.TileContext._drain_fix = True


def _split_excess_waits(nc, limit=1):
    """walrus in this container rejects instructions with more than one
    embedded sync-wait; hoist excess waits onto preceding same-engine nops."""
    n = 0
    for f in nc.m.functions:
        for b in f.blocks:
            insts = b.instructions
            out = []
            changed = False
            for ins in insts:
                si = ins.sync_info
                waits = list(si.on_wait) if si is not None else []
                if len(waits) > limit:
                    changed = True
                    for w in waits[:-limit]:
                        n += 1
                        out.append(mybir.InstNoOp(
                            name=f"waitnop-{n}", engine=ins.engine,
                            bass_nofuse=True,
                            sync_info=mybir.SyncInfo(on_wait=[w],
                                                     on_update=[])))
                    ins.sync_info = mybir.SyncInfo(
                        on_wait=waits[-limit:], on_update=list(si.on_update))
                out.append(ins)
            if changed:
                b.instructions = out
    return n


def build_bass():
    _fix_tile_drain()
    nc = bass.Bass()
    dram = {}
    dram["xT"] = nc.dram_tensor("xT", [D, P], BF16, kind="ExternalInput")
    dram["wpack"] = nc.dram_tensor("wpack", [128, NWC], BF16,
                                   kind="ExternalInput")
    dram["identf"] = nc.dram_tensor("identf", [128, 128], F32,
                                    kind="ExternalInput")
    dram["onesP"] = nc.dram_tensor("onesP", [1, P], BF16,
                                   kind="ExternalInput")
    dram["out"] = nc.dram_tensor("out", [P, D], BF16, kind="ExternalOutput")
    dram["scr_r"] = nc.dram_tensor("scr_r", [P], BF16)
    with tile.TileContext(nc) as tc:
        _emit(nc, tc, dram)
    _split_excess_waits(nc)
    return nc


def _interleave(streams):
    """streams: list of (generator, weight). Round-robin: advance each
    generator up to `weight` steps per round until all are exhausted."""
    live = [[g, w] for g, w in streams]
    while live:
        nxt = []
        for g, w in live:
            alive = True
            for _ in range(w):
                try:
                    next(g)
                except StopIteration:
                    alive = False
                    break
            if alive:
                nxt.append([g, w])
        live = nxt


def _take(lst, n):
    """Generator yielding up to n popped steps from lst (executing them)."""
    for _ in range(n):
        if not lst:
            return
        lst.pop(0)()
        yield


def _emit(nc, tc, dram):
    from contextlib import ExitStack

    with ExitStack() as ctx:
        const = ctx.enter_context(tc.tile_pool(name="const", bufs=1))
        big = ctx.enter_context(tc.tile_pool(name="big", bufs=1))
        work = ctx.enter_context(tc.tile_pool(name="work", bufs=4))

        # ---- constants -----------------------------------------------
        wsb = const.tile([128, NWC], BF16, tag="wsb")
        nc.sync.dma_start(wsb[:], dram["wpack"][:])
        identf = const.tile([128, 128], F32, tag="identf")
        nc.sync.dma_start(identf[:], dram["identf"][:])
        ones_col_b = const.tile([128, 1], BF16, tag="ocb")
        nc.gpsimd.memset(ones_col_b[:], 1.0)
        ones64_f = const.tile([1, 64], F32, tag="o64")
        nc.gpsimd.memset(ones64_f[:], 1.0)
        ones128_b = const.tile([1, 128], BF16, tag="o128")
        nc.gpsimd.memset(ones128_b[:], 1.0)
        epsn_col = const.tile([128, 1], F32, tag="epsn")
        nc.gpsimd.memset(epsn_col[:], EPS_NORM)

        wq8 = [wsb[:, OFF_WQ + kk * 128:OFF_WQ + (kk + 1) * 128]
               for kk in range(8)]
        wk8 = [wsb[:, OFF_WK + kk * 128:OFF_WK + (kk + 1) * 128]
               for kk in range(8)]
        wv8 = [wsb[:, OFF_WV + kk * 128:OFF_WV + (kk + 1) * 128]
               for kk in range(8)]

        # ---- big persistent tiles ------------------------------------
        qT = big.tile([128, P], BF16, tag="qT")
        kT = big.tile([128, P], BF16, tag="kT")
        Vt = big.tile([128, NSUB * 130], BF16, tag="Vt")
        qfT = big.tile([64, P], BF16, tag="qfT")   # rows 16, 48 = ones
        kfT = big.tile([64, P], BF16, tag="kfT")
        catL = big.tile([128, P], BF16, tag="catL")
        catW = big.tile([128, P], BF16, tag="catW")
        r32 = big.tile([128, NSUB], F32, tag="r32")
        r32T = big.tile([8, 512], BF16, tag="r32T")
        rb_all = big.tile([128, P], BF16, tag="rball")

        def vsl(gsub, h):
            base = gsub * 130 + 65 * h
            return Vt[:, base:base + 65]

        # ---- phase A: rmsnorm, q/k/v projections, feature maps -------
        with tc.tile_pool(name="xp", bufs=1) as xp, \
             tc.tile_pool(name="psA", bufs=1, space="PSUM") as psA:
            xt_sb = xp.tile([128, 8 * P], BF16, tag="xt")
            xv = [xt_sb[:, kk * P:(kk + 1) * P] for kk in range(8)]
            for q in range(4):
                qsl = slice(q * 1024, (q + 1) * 1024)
                for kk in range(8):
                    nc.sync.dma_start(xv[kk][:, qsl],
                                      dram["xT"][kk * 128:(kk + 1) * 128,
                                                 qsl])

            # per-quarter pipeline: squares -> r(quarter) -> q/k proj ->
            # V -> feature maps, so nothing waits on a global r barrier
            sq_ps = psA.tile([128, NSUB], F32, tag="sq")
            nc.gpsimd.memset(
                Vt[:].rearrange("p (s h o) -> p s h o", h=2, o=65)[:, :, :, 64],
                1.0)
            for q in range(4):
                qsl = slice(q * 1024, (q + 1) * 1024)
                q8 = slice(q * 8, (q + 1) * 8)
                sqts = []
                for kk in range(8):
                    sqt = work.tile([128, 1024], BF16, tag="sqt", bufs=8)
                    src = xv[kk][:, qsl]
                    if kk % 2 == 0:
                        nc.scalar.activation(sqt[:], src, AF.Square)
                    else:
                        nc.vector.tensor_tensor(sqt[:], src, src, ALU.mult)
                    sqts.append(sqt)
                for sub in range(8):
                    col = q * 8 + sub
                    for kk in range(8):
                        nc.tensor.matmul(
                            sq_ps[:, col:col + 1],
                            sqts[kk][:, sub * 128:(sub + 1) * 128],
                            ones_col_b[:], start=(kk == 0), stop=(kk == 7))
                # r for this quarter: [128, 8] column block, transposed into
                # a [1, 1024] slice of r_row via PE transpose + sbuf DMA
                nc.scalar.activation(r32[:, q8], sq_ps[:, q8], AF.Sqrt,
                                     bias=epsn_col[:], scale=1.0 / D)
                nc.vector.reciprocal(r32[:, q8], r32[:, q8])
                rT_ps = psA.tile([8, 128], F32, tag="rT")
                nc.tensor.transpose(rT_ps[:], r32[:, q8], identf[:])
                rtc = slice(q * 128, (q + 1) * 128)
                nc.scalar.copy(r32T[0:8, rtc], rT_ps[:])
                nc.sync.dma_start(
                    dram["scr_r"][qsl].rearrange("(s c) -> s c", c=128),
                    r32T[0:8, rtc])
                nc.sync.dma_start(
                    rb_all[:, qsl],
                    dram["scr_r"][qsl].unsqueeze(0).broadcast_to((128, 1024)))
                # q/k projections (scaled), then feature maps, this quarter
                for pc in (2 * q, 2 * q + 1):
                    sl = slice(pc * 512, (pc + 1) * 512)
                    for dst, w8 in ((qT, wq8), (kT, wk8)):
                        pj = psA.tile([128, 512], F32, tag="pj", bufs=2)
                        for kk in range(8):
                            nc.tensor.matmul(pj[:], w8[kk], xv[kk][:, sl],
                                             start=(kk == 0), stop=(kk == 7))
                        nc.vector.tensor_tensor(dst[:, sl], pj[:],
                                                rb_all[:, sl], ALU.mult)
                    for i, (dstT, woff, srcT) in enumerate(
                            ((qfT, OFF_WFQ, qT), (kfT, OFF_WFK, kT))):
                        fp = psA.tile([49, 512], F32, tag="rb", bufs=2)
                        nc.tensor.matmul(fp[:], wsb[:, woff:woff + 49],
                                         srcT[:, sl], start=True, stop=True)
                        if (pc + i) % 2 == 0:
                            nc.scalar.copy(dstT[0:49, sl], fp[:])
                        else:
                            nc.vector.tensor_copy(dstT[0:49, sl], fp[:])
                # V for this quarter
                for s in range(q * 8, (q + 1) * 8):
                    sl = slice(s * SUB, (s + 1) * SUB)
                    vp = psA.tile([128, 128], F32, tag="vp", bufs=2)
                    for kk in range(8):
                        nc.tensor.matmul(vp[:], xv[kk][:, sl], wv8[kk],
                                         start=(kk == 0), stop=(kk == 7))
                    base = s * 130
                    dst = Vt[:, base:base + 130].rearrange(
                        "p (h x) -> p h x", x=65)[:, :, 0:64]
                    srcv = vp[:].rearrange("p (h x) -> p h x", x=64)
                    nc.vector.tensor_scalar_mul(dst, srcv, r32[:, s:s + 1])

        nc.sync.dma_start(qfT[16:17, :], dram["onesP"][:])
        nc.sync.dma_start(qfT[48:49, :], dram["onesP"][:])

        # ---- scan + window + out-proj, software-pipelined ------------
        def make_win_step(psW, winw, b, c):
            def step():
                p0 = b * T + c * SUB
                slq = slice(p0, p0 + SUB)
                g = b * (T // SUB) + c
                nblk = 2 if c == 0 else 4
                # separate [128,128] score tiles (one matmul group per psum
                # bank); exp lands in slices of one staging tile so the mask
                # multiply stays batched
                pexp = winw.tile([128, 512], BF16, tag="pexp")
                for h in range(HPC):
                    hd = slice(h * DH, (h + 1) * DH)
                    sbs = (c,) if c == 0 else (c - 1, c)
                    for i, sb in enumerate(sbs):
                        ssl = slice(b * T + sb * SUB, b * T + (sb + 1) * SUB)
                        stt = psW.tile([128, 128], F32, tag="st", bufs=2,
                                       name="stt")
                        nc.tensor.matmul(stt[:], kT[hd, ssl], qT[hd, slq],
                                         start=True, stop=True)
                        blk = (len(sbs) * h + i) * 128
                        nc.scalar.activation(pexp[:, blk:blk + 128], stt[:],
                                             AF.Exp, bias=0.0,
                                             scale=QK_SCALE)
                pexp2 = winw.tile([128, 512], BF16, tag="pexp2")
                if c == 0:
                    mview = wsb[:, OFF_WINM:OFF_WINM + 512].rearrange(
                        "p (a x) -> p a x", x=256)[:, :, 128:256]
                    nc.gpsimd.tensor_tensor(
                        pexp2[:, 0:256].rearrange("p (a x) -> p a x", x=128),
                        pexp[:, 0:256].rearrange("p (a x) -> p a x", x=128),
                        mview, ALU.mult)
                else:
                    nc.vector.tensor_tensor(
                        pexp2[:, 0:256], pexp[:, 0:256],
                        wsb[:, OFF_WINM:OFF_WINM + 256], ALU.mult)
                    nc.gpsimd.tensor_tensor(
                        pexp2[:, 256:512], pexp[:, 256:512],
                        wsb[:, OFF_WINM + 256:OFF_WINM + 512], ALU.mult)
                ytw2 = psW.tile([65, 256], F32, tag="yw", bufs=1)
                for h in range(HPC):
                    if c == 0:
                        nc.tensor.matmul(
                            ytw2[:, h * 128:(h + 1) * 128], vsl(g, h),
                            pexp2[:, h * 128:(h + 1) * 128],
                            start=True, stop=True)
                    else:
                        nc.tensor.matmul(ytw2[:, h * 128:(h + 1) * 128],
                                         vsl(g - 1, h),
                                         pexp2[:, (2 * h) * 128:
                                               (2 * h) * 128 + 128],
                                         start=True, stop=False)
                        nc.tensor.matmul(ytw2[:, h * 128:(h + 1) * 128],
                                         vsl(g, h),
                                         pexp2[:, (2 * h + 1) * 128:
                                               (2 * h + 1) * 128 + 128],
                                         start=False, stop=True)
                ziw = winw.tile([1, 256], F32, tag="ziw", bufs=2)
                nc.vector.reciprocal(ziw[:], ytw2[64:65, :])
                zbw2 = psW.tile([128, 128], F32, tag="yw", bufs=1)
                nc.tensor.matmul(zbw2[0:64, :], ones64_f[:],
                                 ziw[0:1, 0:128], start=True, stop=True)
                nc.tensor.matmul(zbw2[64:128, :], ones64_f[:],
                                 ziw[0:1, 128:256], start=True, stop=True)
                ywsb2 = winw.tile([128, 128], BF16, tag="ywsb", bufs=2)
                nc.scalar.copy(ywsb2[0:64, :], ytw2[0:64, 0:128])
                nc.scalar.copy(ywsb2[64:128, :], ytw2[0:64, 128:256])
                nc.vector.tensor_tensor(catW[:, slq], ywsb2[:], zbw2[:],
                                        ALU.mult)
            return step

        out_shared = {}

        def make_out_step(psO, outw, b, c):
            def step():
                s = b * 16 + c
                sl = slice(s * SUB, (s + 1) * SUB)
                if c % 4 == 0:
                    ob4 = outw.tile([128, 4 * D], BF16, tag="obuf",
                                    bufs=2, name="ob4")
                    out_shared[b] = ob4
                obuf = out_shared[b]
                for hf in range(2):
                    op = psO.tile([128, 512], F32, tag="op", bufs=3)
                    nc.tensor.matmul(op[:], catL[:, sl],
                                     wsb[:, OFF_W1 + hf * 512:
                                         OFF_W1 + (hf + 1) * 512],
                                     start=True, stop=False)
                    nc.tensor.matmul(op[:], catW[:, sl],
                                     wsb[:, OFF_W2 + hf * 512:
                                         OFF_W2 + (hf + 1) * 512],
                                     start=False, stop=True)
                    dsl = slice((c % 4) * D + hf * 512,
                                (c % 4) * D + (hf + 1) * 512)
                    if hf == 0:
                        nc.scalar.copy(obuf[:, dsl], op[:])
                    else:
                        nc.vector.tensor_copy(obuf[:, dsl], op[:])
                if c % 4 == 3:
                    s0 = (s - 3) * SUB
                    dst = dram["out"][s0:s0 + 512, :].rearrange(
                        "(t p) d -> p t d", p=128)
                    nc.sync.dma_start(dst, obuf[:])
            return step

        scan_shared = {}

        def gen_A(b, h, psS, scanw):
            hd = slice(h * DH, (h + 1) * DH)
            hd2 = slice(h * 64, h * 64 + 64)
            qq_all = scanw.tile([128, 16 * 256], BF16, tag="qqall", bufs=2)
            qqT_all = scanw.tile([128, 16 * 256], BF16, tag="qqTall", bufs=2)
            qk_all = scanw.tile([128, 16 * 256], BF16, tag="qkall", bufs=2)
            feat_all = scanw.tile([128, 16 * 34], BF16, tag="ftall",
                                  bufs=2)
            scan_shared[(b, h)] = (qq_all, qqT_all, qk_all, feat_all)
            nc.gpsimd.memset(
                feat_all[:].rearrange("p (t c) -> p t c", c=34)[:, :, 32],
                1.0)
            # all qf/kf projections of this (b,h) into one PSUM tile,
            # then a single strided copy into the feature slab
            qk_ps = psS.tile([128, 512], F32, tag="scr", bufs=2)
            for t in range(16):
                p0 = b * T + t * 128
                sl = slice(p0, p0 + 128)
                nc.tensor.matmul(qk_ps[:, t * 32:t * 32 + 16], qT[hd, sl],
                                 wsb[hd2, OFF_WQF1:OFF_WQF1 + 16],
                                 start=True, stop=True)
                nc.tensor.matmul(qk_ps[:, t * 32 + 16:t * 32 + 32],
                                 kT[hd, sl],
                                 wsb[hd2, OFF_WKF1:OFF_WKF1 + 16],
                                 start=True, stop=True)
            nc.scalar.copy(
                feat_all[:].rearrange("p (t c) -> p t c", c=34)[:, :, 0:32],
                qk_ps[:].rearrange("p (t c) -> p t c", c=32))
            yield
            for sc in range(NSC_B):
                for cb in range(2):
                    t = 2 * sc + cb
                    ft = feat_all[:, t * 34:t * 34 + 34]
                    qv = qq_all[:, t * 256:(t + 1) * 256].rearrange(
                        "p (i j) -> p i j", i=16)
                    kv_ = qk_all[:, t * 256:(t + 1) * 256].rearrange(
                        "p (i j) -> p i j", i=16)
                    qg1 = ft[:, 0:16].unsqueeze(2).broadcast_to(
                        (128, FT, FT))
                    qg2 = ft[:, 0:16].unsqueeze(1).broadcast_to(
                        (128, FT, FT))
                    kg1 = ft[:, 16:32].unsqueeze(2).broadcast_to(
                        (128, FT, FT))
                    kg2 = ft[:, 16:32].unsqueeze(1).broadcast_to(
                        (128, FT, FT))
                    nc.vector.tensor_tensor(qv[:, 0:8, :], qg1[:, 0:8, :],
                                            qg2[:, 0:8, :], ALU.mult)
                    nc.gpsimd.tensor_tensor(qv[:, 8:16, :], qg1[:, 8:16, :],
                                            qg2[:, 8:16, :], ALU.mult)
                    nc.gpsimd.tensor_tensor(kv_[:, 0:8, :], kg1[:, 0:8, :],
                                            kg2[:, 0:8, :], ALU.mult)
                    nc.vector.tensor_tensor(kv_[:, 8:16, :], kg1[:, 8:16, :],
                                            kg2[:, 8:16, :], ALU.mult)
                    yield
            nc.sync.dma_start_transpose(
                qqT_all[:].rearrange("p (t c) -> p t c", c=128), qq_all[:])

        def gen_B(b, h, psS, scanw):
            hd = slice(h * DH, (h + 1) * DH)
            h16 = slice(h * 32, h * 32 + 16)
            h17 = slice(h * 32, h * 32 + 17)
            _, qqT_all, qk_all, feat_all = scan_shared.pop((b, h))
            kv_sb = scanw.tile([128, 196], BF16, tag="kvsb", bufs=2)
            lo0 = 0 if h == 0 else 32
            for sc in range(NSC_B):
                p0 = b * T + sc * SC
                gs0 = p0 // SUB
                sl0 = slice(p0, p0 + 128)
                sl1 = slice(p0 + 128, p0 + 256)
                # intra-chunk score trio: [diag0 | diag1 | off(0->1)]
                strio = psS.tile([128, 384], F32, tag="scr", bufs=2)
                nc.tensor.matmul(strio[:, 0:128], kfT[h16, sl0],
                                 qfT[h16, sl0], start=True, stop=True)
                nc.tensor.matmul(strio[:, 128:256], kfT[h16, sl1],
                                 qfT[h16, sl1], start=True, stop=True)
                nc.tensor.matmul(strio[:, 256:384], kfT[h16, sl0],
                                 qfT[h16, sl1], start=True, stop=True)
                a_all = work.tile([128, 384], BF16, tag="asb", bufs=2)
                nc.scalar.activation(a_all[:], strio[:], AF.Square,
                                     bias=1.0, scale=0.5)
                am = work.tile([128, 256], BF16, tag="am", bufs=2)
                nc.vector.tensor_tensor(
                    am[:], a_all[:, 0:256],
                    wsb[:, OFF_MTRI2:OFF_MTRI2 + 256], ALU.mult)

                yt = psS.tile([65, 256], F32, tag="yt", bufs=2)
                for cb in range(2):
                    sl = sl0 if cb == 0 else sl1
                    t = 2 * sc + cb
                    yv = yt[:, cb * 128:(cb + 1) * 128]
                    ops = []
                    if cb == 0:
                        ops.append((vsl(gs0, h), am[:, 0:128]))
                    else:
                        ops.append((vsl(gs0, h), a_all[:, 256:384]))
                        ops.append((vsl(gs0 + 1, h), am[:, 128:256]))
                    if sc > 0:
                        ops.append((kv_sb[:, 0:65],
                                    qqT_all[:, (2 * t) * 128:
                                            (2 * t) * 128 + 128]))
                        ops.append((kv_sb[:, 65:130],
                                    qqT_all[:, (2 * t + 1) * 128:
                                            (2 * t + 1) * 128 + 128]))
                        ops.append((kv_sb[h17, 130:195], qfT[h17, sl]))
                    for i, (lt, rt) in enumerate(ops):
                        nc.tensor.matmul(yv, lt, rt, start=(i == 0),
                                         stop=(i == len(ops) - 1))

                # state update: per-sc delta (short psum groups), then
                # bf16 state accumulate in SBUF
                if sc < NSC_B - 1:
                    kv = psS.tile([128, 196], F32, tag="kv", bufs=1)
                    t0, t1 = 2 * sc, 2 * sc + 1
                    va0, va1 = vsl(gs0, h), vsl(gs0 + 1, h)
                    for lo, hi, src0, src1 in (
                        (0, 65, qk_all[:, t0 * 256:t0 * 256 + 128],
                         qk_all[:, t1 * 256:t1 * 256 + 128]),
                        (65, 130, qk_all[:, t0 * 256 + 128:(t0 + 1) * 256],
                         qk_all[:, t1 * 256 + 128:(t1 + 1) * 256]),
                    ):
                        nc.tensor.matmul(kv[:, lo:hi], src0, va0,
                                         start=True, stop=False)
                        nc.tensor.matmul(kv[:, lo:hi], src1, va1,
                                         start=False, stop=True)
                    nc.tensor.matmul(kv[lo0:lo0 + 17, 130:195],
                                     feat_all[:, t0 * 34 + 16:t0 * 34 + 33],
                                     va0, start=True, stop=False)
                    nc.tensor.matmul(kv[lo0:lo0 + 17, 130:195],
                                     feat_all[:, t1 * 34 + 16:t1 * 34 + 33],
                                     va1, start=False, stop=True)
                    kq = kv[:, 0:130]
                    klo = kv[lo0:lo0 + 17, 130:195]
                    if sc == 0:
                        nc.vector.tensor_copy(kv_sb[:, 0:130], kq)
                        nc.vector.tensor_copy(kv_sb[h17, 130:195], klo)
                    else:
                        nc.vector.tensor_tensor(kv_sb[:, 0:130],
                                                kv_sb[:, 0:130], kq,
                                                ALU.add)
                        nc.vector.tensor_tensor(kv_sb[h17, 130:195],
                                                kv_sb[h17, 130:195], klo,
                                                ALU.add)

                # normalize into catL
                zi = work.tile([1, 256], F32, tag="zi", bufs=2)
                nc.vector.reciprocal(zi[:], yt[64:65, :])
                zb = psS.tile([64, 256], F32, tag="scr", bufs=2)
                nc.tensor.matmul(zb[:], ones64_f[:], zi[:],
                                 start=True, stop=True)
                zb_sb = work.tile([64, 256], BF16, tag="zbsb", bufs=2)
                nc.scalar.copy(zb_sb[:], zb[:])
                nc.vector.tensor_tensor(
                    catL[hd, slice(p0, p0 + 256)], yt[0:64, :], zb_sb[:],
                    ALU.mult)
                yield

        # drive the pipelined emission. PSUM budget: psS (kv 1 + scr 2
        # + yt 2 = 5 banks) spans everything; psW (st 1 + yw 2 = 3) lives
        # through stage 2 (all windows); psO (op 3) for stages 3-5.
        with tc.tile_pool(name="scanw", bufs=2) as scanw, \
             tc.tile_pool(name="psS", bufs=2, space="PSUM") as psS:
            with tc.tile_pool(name="winw", bufs=2) as winw, \
                 tc.tile_pool(name="psW", bufs=1, space="PSUM") as psW:
                win_steps = [make_win_step(psW, winw, b, c)
                             for b in range(B) for c in range(T // SUB)]
                # stage 0: A(0,0) + 8 win
                _interleave([(gen_A(0, 0, psS, scanw), 2),
                             (_take(win_steps, 8), 1)])
                # stage 1: B(0,0) + A(0,1) + 9 win
                _interleave([(gen_B(0, 0, psS, scanw), 1),
                             (gen_A(0, 1, psS, scanw), 2),
                             (_take(win_steps, 9), 1)])
                # stage 2: B(0,1) + A(1,0) + 9 win
                _interleave([(gen_B(0, 1, psS, scanw), 1),
                             (gen_A(1, 0, psS, scanw), 2),
                             (_take(win_steps, 9), 1)])
                # stage 3: B(1,0) + A(1,1) + rest of win
                _interleave([(gen_B(1, 0, psS, scanw), 1),
                             (gen_A(1, 1, psS, scanw), 2),
                             (_take(win_steps, 6), 1)])
                for stp in win_steps:
                    stp()
                win_steps.clear()
            with tc.tile_pool(name="outw", bufs=2) as outw, \
                 tc.tile_pool(name="psO", bufs=3, space="PSUM") as psO:
                out_b0 = [make_out_step(psO, outw, 0, c) for c in range(16)]
                out_b1 = [make_out_step(psO, outw, 1, c) for c in range(16)]
                # stage 4: B(1,1) + all out(b0) + out(b1) as catL(b1) lands
                tail = []
                for i in range(8):
                    tail.append(out_b0.pop(0))
                    tail.append(out_b0.pop(0))
                    tail.append(out_b1[2 * i])
                    tail.append(out_b1[2 * i + 1])
                _interleave([(gen_B(1, 1, psS, scanw), 1),
                             (_take(tail, len(tail)), 4)])
                for stp in tail:
                    stp()


_NC_CACHE = None
def _get_nc():
    global _NC_CACHE
    if _NC_CACHE is None:
        _NC_CACHE = build_bass()
    return _NC_CACHE


def _host_prep(x, norm_w, Wq, Wk, Wv, Wqf, Wkf, Wout):
    xp = np.ascontiguousarray(x.reshape(P, D).T).astype(BF)
    nw = norm_w.astype(np.float64)
    wq_f = nw[:, None] * Wq.astype(np.float64)
    wk_f = nw[:, None] * Wk.astype(np.float64)
    wv_f = nw[:, None] * Wv.astype(np.float64)

    si = np.arange(128)[:, None]
    ci = np.arange(128)[None, :]
    mtri = (si <= ci).astype(np.float32)
    mwd = ((si <= ci) & (si >= ci - WINDOW)).astype(np.float32)
    mwp = (si >= ci + WINDOW).astype(np.float32)
    mtri2 = np.concatenate([mtri, mtri], 1)
    winm = np.concatenate([mwp, mwd, mwp, mwd], 1)

    sq2 = math.sqrt(2.0)
    wfq = np.zeros((128, 49), np.float32)
    wfq[0:64, 0:16] = Wqf * sq2
    wfq[64:128, 32:48] = Wqf * sq2
    wfk = np.zeros((128, 49), np.float32)
    wfk[0:64, 0:16] = Wkf / sq2
    wfk[64:128, 32:48] = Wkf / sq2
    wqf1 = np.vstack([Wqf, Wqf]) / sq2
    wkf1 = np.vstack([Wkf, Wkf]) / sq2

    in_maps = []
    for c in range(NCORES):
        csl = slice(c * 128, (c + 1) * 128)
        wq_sb = wq_f[:, csl].reshape(8, 128, 128).transpose(1, 0, 2).reshape(
            128, 1024)
        wk_sb = wk_f[:, csl].reshape(8, 128, 128).transpose(1, 0, 2).reshape(
            128, 1024)
        wv_sb = wv_f[:, csl].reshape(8, 128, 128).transpose(1, 0, 2).reshape(
            128, 1024)
        wpack = np.zeros((128, NWC), np.float32)
        wpack[:, OFF_WQ:OFF_WQ + 1024] = wq_sb
        wpack[:, OFF_WK:OFF_WK + 1024] = wk_sb
        wpack[:, OFF_WV:OFF_WV + 1024] = wv_sb
        wpack[:, OFF_W1:OFF_W1 + 1024] = Wout[csl, :]
        wpack[:, OFF_W2:OFF_W2 + 1024] = Wout[1024 + c * 128:
                                              1024 + (c + 1) * 128, :]
        wpack[:, OFF_WFQ:OFF_WFQ + 49] = wfq
        wpack[:, OFF_WFK:OFF_WFK + 49] = wfk
        wpack[:, OFF_WQF1:OFF_WQF1 + 16] = wqf1
        wpack[:, OFF_WKF1:OFF_WKF1 + 16] = wkf1
        wpack[:, OFF_MTRI2:OFF_MTRI2 + 256] = mtri2
        wpack[:, OFF_WINM:OFF_WINM + 512] = winm
        in_maps.append({
            "xT": xp,
            "wpack": wpack.astype(BF),
            "identf": np.eye(128, dtype=np.float32),
            "onesP": np.ones((1, P), np.float32).astype(BF),
        })
    return in_maps


def kernel(x, norm_w, Wq, Wk, Wv, Wqf, Wkf, Wout) -> np.ndarray:
    x = np.asarray(x, np.float32)
    in_maps = _host_prep(
        x, np.asarray(norm_w, np.float32), np.asarray(Wq, np.float32),
        np.asarray(Wk, np.float32), np.asarray(Wv, np.float32),
        np.asarray(Wqf, np.float32), np.asarray(Wkf, np.float32),
        np.asarray(Wout, np.float32))
    nc = _get_nc()
    res = run_bass_kernel_spmd(nc, in_maps, list(range(NCORES)))
    acc = np.zeros((P, D), np.float32)
    for c in range(NCORES):
        acc += res.results[c]["out"].astype(np.float32)
    return (x.reshape(P, D) + acc).reshape(B, T, D).astype(np.float32)


# revision 36
# speedup vs baseline: 1.3254x; 1.0262x over previous
"""BasedAttention Trainium2 kernel — nn_BasedAttention_82214263980185.

Head-sharded across 8 NeuronCores (2 heads/core): column-parallel QKV,
per-head taylor linear attention (factorized phi) + banded sliding-window
attention, row-parallel out-proj with host-side partial reduction.

v3: batched DMAs, quarter-granular x loads overlapped with squares,
single batched block-transpose per (b,h) for quad features, 1-col matmul
rmsnorm reduction, merged per-chunk PSUM tiles (one reciprocal / one
normalize per 256 positions), software-pipelined scan emission
(B(b,h) overlapped with A(next)) with window / out-proj filler steps.

Math notes:
  - reference phi(x) = [1, x, tri-scaled quad] gives
    phi(q).phi(k) = 1 + s + 0.25 s^2  (s = qf.kf).  We use the equivalent
    full-outer 256-feature quad block scaled 2^-0.25 per side plus
    [x, ones]: identical inner products, rectangular construction.
  - Intra-chunk scores: A = (1 + 0.5 s)^2 = 1 + s + 0.25 s^2 directly.
  - rmsnorm: norm_w folds into QKV weights on host; the per-row 1/rms
    factor r applies to q, k, v after projection (all linear in r).
"""

import math
import os
import sys

for _p in ("/opt/trn_rl_repo",):
    if _p not in sys.path:
        sys.path.insert(0, _p)

import numpy as np
import ml_dtypes

import concourse.bass as bass
import concourse.mybir as mybir
import concourse.tile as tile
from concourse.bass_utils import run_bass_kernel_spmd

F32 = mybir.dt.float32
BF16 = mybir.dt.bfloat16
AF = mybir.ActivationFunctionType
ALU = mybir.AluOpType
BF = ml_dtypes.bfloat16

B, T, D = 2, 2048, 1024
P = B * T          # 4096 positions
NH, DH, FT = 16, 64, 16
HPC = 2            # heads per core
NCORES = 8
WINDOW = 64
EPS_NORM = 1e-6
SUB = 128          # position sub-chunk (partition tile)
NSUB = P // SUB    # 32
SC = 256           # linear-attention scan chunk
NSC_B = T // SC    # 8 scan chunks per (b,h) sequence
QK_SCALE = 1.0 / math.sqrt(DH)

# weight-pack column offsets (bf16, [128, NWC])
OFF_WQ = 0
OFF_WK = 1024
OFF_WV = 2048
OFF_W1 = 3072
OFF_W2 = 4096
OFF_WFQ = 5120       # [128, 49]
OFF_WFK = 5169       # [128, 49]
OFF_WQF1 = 5218      # [128, 16]
OFF_WKF1 = 5234      # [128, 16]
OFF_MTRI2 = 5250     # [128, 256] = [tril | tril]
OFF_WINM = 5506      # [128, 512] = [mwp | mwd | mwp | mwd]
NWC = 6018


def _fix_tile_drain():
    """walrus here accepts only 1 sync-wait on the Tile tail drain; spread
    the global-clock waits over sequencer nop carriers."""
    if getattr(tile.TileContext, "_drain_fix", False):
        return
    from concourse.tile import ScopedClock

    def _patched(self, tick_clock, wait_clock):
        nc = self.nc
        carriers = [nc.sync.nop(nofuse=True) for _ in range(30)]
        drain_inst = nc.sync.drain()
        wait_clock.add_sem_waits(
            drain_inst.ins, ScopedClock({None: tick_clock.global_clock})
        )
        si = drain_inst.ins.sync_info
        waits = list(si.on_wait) if si is not None else []
        if len(waits) > 1:
            keep, rest = waits[:1], waits[1:]
            assert len(rest) <= len(carriers), f"too many waits: {len(waits)}"
            for c, w in zip(carriers, rest):
                c.ins.sync_info = mybir.SyncInfo(on_wait=[w], on_update=[])
            drain_inst.ins.sync_info = mybir.SyncInfo(
                on_wait=keep, on_update=list(si.on_update)
            )
        nc.all_engine_barrier()
        assert self.sems is not None
        popped = nc._tile_sem_poison_stack.pop()
        assert popped is self._sem_poison
        nc.clear_and_free_semaphores(list(self.sems.allocated().values()))
        nc.all_engine_barrier()

    tile.TileContext._drain_and_barrier = _patched
    tile.TileContext._drain_fix = True


def _split_excess_waits(nc, limit=1):
    """walrus in this container rejects instructions with more than one
    embedded sync-wait; hoist excess waits onto preceding same-engine nops."""
    n = 0
    for f in nc.m.functions:
        for b in f.blocks:
            insts = b.instructions
            out = []
            changed = False
            for ins in insts:
                si = ins.sync_info
                waits = list(si.on_wait) if si is not None else []
                if len(waits) > limit:
                    changed = True
                    for w in waits[:-limit]:
                        n += 1
                        out.append(mybir.InstNoOp(
                            name=f"waitnop-{n}", engine=ins.engine,
                            bass_nofuse=True,
                            sync_info=mybir.SyncInfo(on_wait=[w],
                                                     on_update=[])))
                    ins.sync_info = mybir.SyncInfo(
                        on_wait=waits[-limit:], on_update=list(si.on_update))
                out.append(ins)
            if changed:
                b.instructions = out
    return n


def build_bass():
    _fix_tile_drain()
    nc = bass.Bass()
    dram = {}
    dram["xT"] = nc.dram_tensor("xT", [D, P], BF16, kind="ExternalInput")
    dram["wpack"] = nc.dram_tensor("wpack", [128, NWC], BF16,
                                   kind="ExternalInput")
    dram["identf"] = nc.dram_tensor("identf", [128, 128], F32,
                                    kind="ExternalInput")
    dram["onesP"] = nc.dram_tensor("onesP", [1, P], BF16,
                                   kind="ExternalInput")
    dram["out"] = nc.dram_tensor("out", [P, D], BF16, kind="ExternalOutput")
    dram["scr_r"] = nc.dram_tensor("scr_r", [P], BF16)
    with tile.TileContext(nc) as tc:
        _emit(nc, tc, dram)
    _split_excess_waits(nc)
    return nc


def _interleave(streams):
    """streams: list of (generator, weight). Round-robin: advance each
    generator up to `weight` steps per round until all are exhausted."""
    live = [[g, w] for g, w in streams]
    while live:
        nxt = []
        for g, w in live:
            alive = True
            for _ in range(w):
                try:
                    next(g)
                except StopIteration:
                    alive = False
                    break
            if alive:
                nxt.append([g, w])
        live = nxt


def _take(lst, n):
    """Generator yielding up to n popped steps from lst (executing them)."""
    for _ in range(n):
        if not lst:
            return
        lst.pop(0)()
        yield


def _emit(nc, tc, dram):
    from contextlib import ExitStack

    with ExitStack() as ctx:
        const = ctx.enter_context(tc.tile_pool(name="const", bufs=1))
        big = ctx.enter_context(tc.tile_pool(name="big", bufs=1))
        work = ctx.enter_context(tc.tile_pool(name="work", bufs=4))

        # ---- constants -----------------------------------------------
        wsb = const.tile([128, NWC], BF16, tag="wsb")
        nc.sync.dma_start(wsb[:], dram["wpack"][:])
        identf = const.tile([128, 128], F32, tag="identf")
        nc.sync.dma_start(identf[:], dram["identf"][:])
        ones_col_b = const.tile([128, 1], BF16, tag="ocb")
        nc.gpsimd.memset(ones_col_b[:], 1.0)
        ones64_f = const.tile([1, 64], F32, tag="o64")
        nc.gpsimd.memset(ones64_f[:], 1.0)
        ones128_b = const.tile([1, 128], BF16, tag="o128")
        nc.gpsimd.memset(ones128_b[:], 1.0)
        epsn_col = const.tile([128, 1], F32, tag="epsn")
        nc.gpsimd.memset(epsn_col[:], EPS_NORM)

        wq8 = [wsb[:, OFF_WQ + kk * 128:OFF_WQ + (kk + 1) * 128]
               for kk in range(8)]
        wk8 = [wsb[:, OFF_WK + kk * 128:OFF_WK + (kk + 1) * 128]
               for kk in range(8)]
        wv8 = [wsb[:, OFF_WV + kk * 128:OFF_WV + (kk + 1) * 128]
               for kk in range(8)]

        # ---- big persistent tiles ------------------------------------
        qT = big.tile([128, P], BF16, tag="qT")
        kT = big.tile([128, P], BF16, tag="kT")
        Vt = big.tile([128, NSUB * 130], BF16, tag="Vt")
        qfT = big.tile([64, P], BF16, tag="qfT")   # rows 16, 48 = ones
        kfT = big.tile([64, P], BF16, tag="kfT")
        catL = big.tile([128, P], BF16, tag="catL")
        catW = big.tile([128, P], BF16, tag="catW")
        r32 = big.tile([128, NSUB], F32, tag="r32")
        r32T = big.tile([8, 512], BF16, tag="r32T")
        rb_all = big.tile([128, P], BF16, tag="rball")

        def vsl(gsub, h):
            base = gsub * 130 + 65 * h
            return Vt[:, base:base + 65]

        # ---- phase A: rmsnorm, q/k/v projections, feature maps -------
        with tc.tile_pool(name="xp", bufs=1) as xp, \
             tc.tile_pool(name="psA", bufs=1, space="PSUM") as psA:
            xt_sb = xp.tile([128, 8 * P], BF16, tag="xt")
            xv = [xt_sb[:, kk * P:(kk + 1) * P] for kk in range(8)]
            for q in range(4):
                qsl = slice(q * 1024, (q + 1) * 1024)
                for kk in range(8):
                    nc.sync.dma_start(xv[kk][:, qsl],
                                      dram["xT"][kk * 128:(kk + 1) * 128,
                                                 qsl])

            # per-quarter pipeline: squares -> r(quarter) -> q/k proj ->
            # V -> feature maps, so nothing waits on a global r barrier
            sq_ps = psA.tile([128, NSUB], F32, tag="sq")
            nc.gpsimd.memset(
                Vt[:].rearrange("p (s h o) -> p s h o", h=2, o=65)[:, :, :, 64],
                1.0)
            for q in range(4):
                qsl = slice(q * 1024, (q + 1) * 1024)
                q8 = slice(q * 8, (q + 1) * 8)
                sqts = []
                for kk in range(8):
                    sqt = work.tile([128, 1024], BF16, tag="sqt", bufs=8)
                    src = xv[kk][:, qsl]
                    if kk % 2 == 0:
                        nc.scalar.activation(sqt[:], src, AF.Square)
                    else:
                        nc.vector.tensor_tensor(sqt[:], src, src, ALU.mult)
                    sqts.append(sqt)
                for sub in range(8):
                    col = q * 8 + sub
                    for kk in range(8):
                        nc.tensor.matmul(
                            sq_ps[:, col:col + 1],
                            sqts[kk][:, sub * 128:(sub + 1) * 128],
                            ones_col_b[:], start=(kk == 0), stop=(kk == 7))
                # r for this quarter: [128, 8] column block, transposed into
                # a [1, 1024] slice of r_row via PE transpose + sbuf DMA
                nc.scalar.activation(r32[:, q8], sq_ps[:, q8], AF.Sqrt,
                                     bias=epsn_col[:], scale=1.0 / D)
                nc.vector.reciprocal(r32[:, q8], r32[:, q8])
                rT_ps = psA.tile([8, 128], F32, tag="rT")
                nc.tensor.transpose(rT_ps[:], r32[:, q8], identf[:])
                rtc = slice(q * 128, (q + 1) * 128)
                nc.scalar.copy(r32T[0:8, rtc], rT_ps[:])
                nc.sync.dma_start(
                    dram["scr_r"][qsl].rearrange("(s c) -> s c", c=128),
                    r32T[0:8, rtc])
                nc.sync.dma_start(
                    rb_all[:, qsl],
                    dram["scr_r"][qsl].unsqueeze(0).broadcast_to((128, 1024)))
                # q/k projections (scaled), then feature maps, this quarter
                for pc in (2 * q, 2 * q + 1):
                    sl = slice(pc * 512, (pc + 1) * 512)
                    for dst, w8 in ((qT, wq8), (kT, wk8)):
                        pj = psA.tile([128, 512], F32, tag="pj", bufs=2)
                        for kk in range(8):
                            nc.tensor.matmul(pj[:], w8[kk], xv[kk][:, sl],
                                             start=(kk == 0), stop=(kk == 7))
                        nc.vector.tensor_tensor(dst[:, sl], pj[:],
                                                rb_all[:, sl], ALU.mult)
                    for i, (dstT, woff, srcT) in enumerate(
                            ((qfT, OFF_WFQ, qT), (kfT, OFF_WFK, kT))):
                        fp = psA.tile([49, 512], F32, tag="rb", bufs=2)
                        nc.tensor.matmul(fp[:], wsb[:, woff:woff + 49],
                                         srcT[:, sl], start=True, stop=True)
                        if (pc + i) % 2 == 0:
                            nc.scalar.copy(dstT[0:49, sl], fp[:])
                        else:
                            nc.vector.tensor_copy(dstT[0:49, sl], fp[:])
                # V for this quarter
                for s in range(q * 8, (q + 1) * 8):
                    sl = slice(s * SUB, (s + 1) * SUB)
                    vp = psA.tile([128, 128], F32, tag="vp", bufs=2)
                    for kk in range(8):
                        nc.tensor.matmul(vp[:], xv[kk][:, sl], wv8[kk],
                                         start=(kk == 0), stop=(kk == 7))
                    base = s * 130
                    dst = Vt[:, base:base + 130].rearrange(
                        "p (h x) -> p h x", x=65)[:, :, 0:64]
                    srcv = vp[:].rearrange("p (h x) -> p h x", x=64)
                    nc.vector.tensor_scalar_mul(dst, srcv, r32[:, s:s + 1])

        nc.sync.dma_start(qfT[16:17, :], dram["onesP"][:])
        nc.sync.dma_start(qfT[48:49, :], dram["onesP"][:])

        # ---- scan + window + out-proj, software-pipelined ------------
        def make_win_step(psW, winw, b, c):
            def step():
                p0 = b * T + c * SUB
                slq = slice(p0, p0 + SUB)
                g = b * (T // SUB) + c
                nblk = 2 if c == 0 else 4
                # separate [128,128] score tiles (one matmul group per psum
                # bank); exp lands in slices of one staging tile so the mask
                # multiply stays batched
                pexp = winw.tile([128, 512], BF16, tag="pexp")
                for h in range(HPC):
                    hd = slice(h * DH, (h + 1) * DH)
                    sbs = (c,) if c == 0 else (c - 1, c)
                    for i, sb in enumerate(sbs):
                        ssl = slice(b * T + sb * SUB, b * T + (sb + 1) * SUB)
                        stt = psW.tile([128, 128], F32, tag="st", bufs=2,
                                       name="stt")
                        nc.tensor.matmul(stt[:], kT[hd, ssl], qT[hd, slq],
                                         start=True, stop=True)
                        blk = (len(sbs) * h + i) * 128
                        nc.scalar.activation(pexp[:, blk:blk + 128], stt[:],
                                             AF.Exp, bias=0.0,
                                             scale=QK_SCALE)
                pexp2 = winw.tile([128, 512], BF16, tag="pexp2")
                if c == 0:
                    mview = wsb[:, OFF_WINM:OFF_WINM + 512].rearrange(
                        "p (a x) -> p a x", x=256)[:, :, 128:256]
                    nc.gpsimd.tensor_tensor(
                        pexp2[:, 0:256].rearrange("p (a x) -> p a x", x=128),
                        pexp[:, 0:256].rearrange("p (a x) -> p a x", x=128),
                        mview, ALU.mult)
                else:
                    nc.vector.tensor_tensor(
                        pexp2[:, 0:256], pexp[:, 0:256],
                        wsb[:, OFF_WINM:OFF_WINM + 256], ALU.mult)
                    nc.gpsimd.tensor_tensor(
                        pexp2[:, 256:512], pexp[:, 256:512],
                        wsb[:, OFF_WINM + 256:OFF_WINM + 512], ALU.mult)
                ytw2 = psW.tile([65, 256], F32, tag="yw", bufs=1)
                for h in range(HPC):
                    if c == 0:
                        nc.tensor.matmul(
                            ytw2[:, h * 128:(h + 1) * 128], vsl(g, h),
                            pexp2[:, h * 128:(h + 1) * 128],
                            start=True, stop=True)
                    else:
                        nc.tensor.matmul(ytw2[:, h * 128:(h + 1) * 128],
                                         vsl(g - 1, h),
                                         pexp2[:, (2 * h) * 128:
                                               (2 * h) * 128 + 128],
                                         start=True, stop=False)
                        nc.tensor.matmul(ytw2[:, h * 128:(h + 1) * 128],
                                         vsl(g, h),
                                         pexp2[:, (2 * h + 1) * 128:
                                               (2 * h + 1) * 128 + 128],
                                         start=False, stop=True)
                ziw = winw.tile([1, 256], F32, tag="ziw", bufs=2)
                nc.vector.reciprocal(ziw[:], ytw2[64:65, :])
                zbw2 = psW.tile([128, 128], F32, tag="yw", bufs=1)
                nc.tensor.matmul(zbw2[0:64, :], ones64_f[:],
                                 ziw[0:1, 0:128], start=True, stop=True)
                nc.tensor.matmul(zbw2[64:128, :], ones64_f[:],
                                 ziw[0:1, 128:256], start=True, stop=True)
                ywsb2 = winw.tile([128, 128], BF16, tag="ywsb", bufs=2)
                nc.scalar.copy(ywsb2[0:64, :], ytw2[0:64, 0:128])
                nc.scalar.copy(ywsb2[64:128, :], ytw2[0:64, 128:256])
                nc.vector.tensor_tensor(catW[:, slq], ywsb2[:], zbw2[:],
                                        ALU.mult)
            return step

        out_shared = {}

        def make_out_step(psO, outw, b, c):
            def step():
                s = b * 16 + c
                sl = slice(s * SUB, (s + 1) * SUB)
                if c % 4 == 0:
                    ob4 = outw.tile([128, 4 * D], BF16, tag="obuf",
                                    bufs=2, name="ob4")
                    out_shared[b] = ob4
                obuf = out_shared[b]
                for hf in range(2):
                    op = psO.tile([128, 512], F32, tag="op", bufs=3)
                    nc.tensor.matmul(op[:], catL[:, sl],
                                     wsb[:, OFF_W1 + hf * 512:
                                         OFF_W1 + (hf + 1) * 512],
                                     start=True, stop=False)
                    nc.tensor.matmul(op[:], catW[:, sl],
                                     wsb[:, OFF_W2 + hf * 512:
                                         OFF_W2 + (hf + 1) * 512],
                                     start=False, stop=True)
                    dsl = slice((c % 4) * D + hf * 512,
                                (c % 4) * D + (hf + 1) * 512)
                    if hf == 0:
                        nc.scalar.copy(obuf[:, dsl], op[:])
                    else:
                        nc.vector.tensor_copy(obuf[:, dsl], op[:])
                if c % 4 == 3:
                    s0 = (s - 3) * SUB
                    dst = dram["out"][s0:s0 + 512, :].rearrange(
                        "(t p) d -> p t d", p=128)
                    nc.sync.dma_start(dst, obuf[:])
            return step

        scan_shared = {}

        def gen_A(b, h, psS, scanw):
            hd = slice(h * DH, (h + 1) * DH)
            hd2 = slice(h * 64, h * 64 + 64)
            qq_all = scanw.tile([128, 16 * 256], BF16, tag="qqall", bufs=2)
            qqT_all = scanw.tile([128, 16 * 256], BF16, tag="qqTall", bufs=2)
            qk_all = scanw.tile([128, 16 * 256], BF16, tag="qkall", bufs=2)
            feat_all = scanw.tile([128, 16 * 34], BF16, tag="ftall",
                                  bufs=2)
            scan_shared[(b, h)] = (qq_all, qqT_all, qk_all, feat_all)
            nc.gpsimd.memset(
                feat_all[:].rearrange("p (t c) -> p t c", c=34)[:, :, 32],
                1.0)
            # all qf/kf projections of this (b,h) into one PSUM tile,
            # then a single strided copy into the feature slab
            qk_ps = psS.tile([128, 512], F32, tag="scr", bufs=2)
            for t in range(16):
                p0 = b * T + t * 128
                sl = slice(p0, p0 + 128)
                nc.tensor.matmul(qk_ps[:, t * 32:t * 32 + 16], qT[hd, sl],
                                 wsb[hd2, OFF_WQF1:OFF_WQF1 + 16],
                                 start=True, stop=True)
                nc.tensor.matmul(qk_ps[:, t * 32 + 16:t * 32 + 32],
                                 kT[hd, sl],
                                 wsb[hd2, OFF_WKF1:OFF_WKF1 + 16],
                                 start=True, stop=True)
            fview = feat_all[:].rearrange("p (t c) -> p t c", c=34)
            qview = qk_ps[:].rearrange("p (t c) -> p t c", c=32)
            nc.scalar.copy(fview[:, 0:8, 0:32], qview[:, 0:8, :])
            nc.scalar.copy(fview[:, 8:16, 0:32], qview[:, 8:16, :])
            yield
            for sc in range(NSC_B):
                for cb in range(2):
                    t = 2 * sc + cb
                    ft = feat_all[:, t * 34:t * 34 + 34]
                    qv = qq_all[:, t * 256:(t + 1) * 256].rearrange(
                        "p (i j) -> p i j", i=16)
                    kv_ = qk_all[:, t * 256:(t + 1) * 256].rearrange(
                        "p (i j) -> p i j", i=16)
                    qg1 = ft[:, 0:16].unsqueeze(2).broadcast_to(
                        (128, FT, FT))
                    qg2 = ft[:, 0:16].unsqueeze(1).broadcast_to(
                        (128, FT, FT))
                    kg1 = ft[:, 16:32].unsqueeze(2).broadcast_to(
                        (128, FT, FT))
                    kg2 = ft[:, 16:32].unsqueeze(1).broadcast_to(
                        (128, FT, FT))
                    nc.vector.tensor_tensor(qv[:, 0:8, :], qg1[:, 0:8, :],
                                            qg2[:, 0:8, :], ALU.mult)
                    nc.gpsimd.tensor_tensor(qv[:, 8:16, :], qg1[:, 8:16, :],
                                            qg2[:, 8:16, :], ALU.mult)
                    nc.gpsimd.tensor_tensor(kv_[:, 0:8, :], kg1[:, 0:8, :],
                                            kg2[:, 0:8, :], ALU.mult)
                    nc.vector.tensor_tensor(kv_[:, 8:16, :], kg1[:, 8:16, :],
                                            kg2[:, 8:16, :], ALU.mult)
                    yield
            nc.sync.dma_start_transpose(
                qqT_all[:].rearrange("p (t c) -> p t c", c=128), qq_all[:])

        def gen_B(b, h, psS, scanw):
            hd = slice(h * DH, (h + 1) * DH)
            h16 = slice(h * 32, h * 32 + 16)
            h17 = slice(h * 32, h * 32 + 17)
            _, qqT_all, qk_all, feat_all = scan_shared.pop((b, h))
            kv_sb = scanw.tile([128, 196], BF16, tag="kvsb", bufs=2)
            lo0 = 0 if h == 0 else 32
            for sc in range(NSC_B):
                p0 = b * T + sc * SC
                gs0 = p0 // SUB
                sl0 = slice(p0, p0 + 128)
                sl1 = slice(p0 + 128, p0 + 256)
                # intra-chunk score trio: [diag0 | diag1 | off(0->1)]
                strio = psS.tile([128, 384], F32, tag="scr", bufs=2)
                nc.tensor.matmul(strio[:, 0:128], kfT[h16, sl0],
                                 qfT[h16, sl0], start=True, stop=True)
                nc.tensor.matmul(strio[:, 128:256], kfT[h16, sl1],
                                 qfT[h16, sl1], start=True, stop=True)
                nc.tensor.matmul(strio[:, 256:384], kfT[h16, sl0],
                                 qfT[h16, sl1], start=True, stop=True)
                # state update: per-sc delta (short psum groups), then
                # bf16 state accumulate in SBUF
                if sc < NSC_B - 1:
                    kv = psS.tile([128, 196], F32, tag="kv", bufs=1)
                    t0, t1 = 2 * sc, 2 * sc + 1
                    va0, va1 = vsl(gs0, h), vsl(gs0 + 1, h)
                    for lo, hi, src0, src1 in (
                        (0, 65, qk_all[:, t0 * 256:t0 * 256 + 128],
                         qk_all[:, t1 * 256:t1 * 256 + 128]),
                        (65, 130, qk_all[:, t0 * 256 + 128:(t0 + 1) * 256],
                         qk_all[:, t1 * 256 + 128:(t1 + 1) * 256]),
                    ):
                        nc.tensor.matmul(kv[:, lo:hi], src0, va0,
                                         start=True, stop=False)
                        nc.tensor.matmul(kv[:, lo:hi], src1, va1,
                                         start=False, stop=True)
                    nc.tensor.matmul(kv[lo0:lo0 + 17, 130:195],
                                     feat_all[:, t0 * 34 + 16:t0 * 34 + 33],
                                     va0, start=True, stop=False)
                    nc.tensor.matmul(kv[lo0:lo0 + 17, 130:195],
                                     feat_all[:, t1 * 34 + 16:t1 * 34 + 33],
                                     va1, start=False, stop=True)
                a_all = work.tile([128, 384], BF16, tag="asb", bufs=2)
                nc.scalar.activation(a_all[:], strio[:], AF.Square,
                                     bias=1.0, scale=0.5)
                am = work.tile([128, 256], BF16, tag="am", bufs=2)
                nc.vector.tensor_tensor(
                    am[:], a_all[:, 0:256],
                    wsb[:, OFF_MTRI2:OFF_MTRI2 + 256], ALU.mult)

                yt = psS.tile([65, 256], F32, tag="yt", bufs=2)
                for cb in range(2):
                    sl = sl0 if cb == 0 else sl1
                    t = 2 * sc + cb
                    yv = yt[:, cb * 128:(cb + 1) * 128]
                    ops = []
                    if sc > 0:
                        ops.append((kv_sb[:, 0:65],
                                    qqT_all[:, (2 * t) * 128:
                                            (2 * t) * 128 + 128]))
                        ops.append((kv_sb[:, 65:130],
                                    qqT_all[:, (2 * t + 1) * 128:
                                            (2 * t + 1) * 128 + 128]))
                        ops.append((kv_sb[h17, 130:195], qfT[h17, sl]))
                    if cb == 0:
                        ops.append((vsl(gs0, h), am[:, 0:128]))
                    else:
                        ops.append((vsl(gs0, h), a_all[:, 256:384]))
                        ops.append((vsl(gs0 + 1, h), am[:, 128:256]))
                    for i, (lt, rt) in enumerate(ops):
                        nc.tensor.matmul(yv, lt, rt, start=(i == 0),
                                         stop=(i == len(ops) - 1))

                if sc < NSC_B - 1:
                    kq = kv[:, 0:130]
                    klo = kv[lo0:lo0 + 17, 130:195]
                    if sc == 0:
                        nc.vector.tensor_copy(kv_sb[:, 0:130], kq)
                        nc.vector.tensor_copy(kv_sb[h17, 130:195], klo)
                    else:
                        nc.vector.tensor_tensor(kv_sb[:, 0:130],
                                                kv_sb[:, 0:130], kq,
                                                ALU.add)
                        nc.vector.tensor_tensor(kv_sb[h17, 130:195],
                                                kv_sb[h17, 130:195], klo,
                                                ALU.add)

                # normalize into catL
                zi = work.tile([1, 256], F32, tag="zi", bufs=2)
                nc.vector.reciprocal(zi[:], yt[64:65, :])
                zb = psS.tile([64, 256], F32, tag="scr", bufs=2)
                nc.tensor.matmul(zb[:], ones64_f[:], zi[:],
                                 start=True, stop=True)
                zb_sb = work.tile([64, 256], BF16, tag="zbsb", bufs=2)
                nc.scalar.copy(zb_sb[:], zb[:])
                nc.vector.tensor_tensor(
                    catL[hd, slice(p0, p0 + 256)], yt[0:64, :], zb_sb[:],
                    ALU.mult)
                yield

        # drive the pipelined emission. PSUM budget: psS (kv 1 + scr 2
        # + yt 2 = 5 banks) spans everything; psW (st 1 + yw 2 = 3) lives
        # through stage 2 (all windows); psO (op 3) for stages 3-5.
        with tc.tile_pool(name="scanw", bufs=2) as scanw, \
             tc.tile_pool(name="psS", bufs=2, space="PSUM") as psS:
            with tc.tile_pool(name="winw", bufs=2) as winw, \
                 tc.tile_pool(name="psW", bufs=1, space="PSUM") as psW:
                win_steps = [make_win_step(psW, winw, b, c)
                             for b in range(B) for c in range(T // SUB)]
                # stage 0: A(0,0) + 8 win
                _interleave([(gen_A(0, 0, psS, scanw), 2),
                             (_take(win_steps, 8), 1)])
                # stage 1: B(0,0) + A(0,1) + 9 win
                _interleave([(gen_B(0, 0, psS, scanw), 1),
                             (gen_A(0, 1, psS, scanw), 2),
                             (_take(win_steps, 9), 1)])
                # stage 2: B(0,1) + A(1,0) + 9 win
                _interleave([(gen_B(0, 1, psS, scanw), 1),
                             (gen_A(1, 0, psS, scanw), 2),
                             (_take(win_steps, 9), 1)])
                # stage 3: B(1,0) + A(1,1) + rest of win
                _interleave([(gen_B(1, 0, psS, scanw), 1),
                             (gen_A(1, 1, psS, scanw), 2),
                             (_take(win_steps, 6), 1)])
                for stp in win_steps:
                    stp()
                win_steps.clear()
            with tc.tile_pool(name="outw", bufs=2) as outw, \
                 tc.tile_pool(name="psO", bufs=3, space="PSUM") as psO:
                out_b0 = [make_out_step(psO, outw, 0, c) for c in range(16)]
                out_b1 = [make_out_step(psO, outw, 1, c) for c in range(16)]
                # stage 4: B(1,1) + all out(b0) + out(b1) as catL(b1) lands
                tail = []
                for i in range(8):
                    tail.append(out_b0.pop(0))
                    tail.append(out_b0.pop(0))
                    tail.append(out_b1[2 * i])
                    tail.append(out_b1[2 * i + 1])
                _interleave([(gen_B(1, 1, psS, scanw), 1),
                             (_take(tail, len(tail)), 4)])
                for stp in tail:
                    stp()


_NC_CACHE = None
def _get_nc():
    global _NC_CACHE
    if _NC_CACHE is None:
        _NC_CACHE = build_bass()
    return _NC_CACHE


def _host_prep(x, norm_w, Wq, Wk, Wv, Wqf, Wkf, Wout):
    xp = np.ascontiguousarray(x.reshape(P, D).T).astype(BF)
    nw = norm_w.astype(np.float64)
    wq_f = nw[:, None] * Wq.astype(np.float64)
    wk_f = nw[:, None] * Wk.astype(np.float64)
    wv_f = nw[:, None] * Wv.astype(np.float64)

    si = np.arange(128)[:, None]
    ci = np.arange(128)[None, :]
    mtri = (si <= ci).astype(np.float32)
    mwd = ((si <= ci) & (si >= ci - WINDOW)).astype(np.float32)
    mwp = (si >= ci + WINDOW).astype(np.float32)
    mtri2 = np.concatenate([mtri, mtri], 1)
    winm = np.concatenate([mwp, mwd, mwp, mwd], 1)

    sq2 = math.sqrt(2.0)
    wfq = np.zeros((128, 49), np.float32)
    wfq[0:64, 0:16] = Wqf * sq2
    wfq[64:128, 32:48] = Wqf * sq2
    wfk = np.zeros((128, 49), np.float32)
    wfk[0:64, 0:16] = Wkf / sq2
    wfk[64:128, 32:48] = Wkf / sq2
    wqf1 = np.vstack([Wqf, Wqf]) / sq2
    wkf1 = np.vstack([Wkf, Wkf]) / sq2

    in_maps = []
    for c in range(NCORES):
        csl = slice(c * 128, (c + 1) * 128)
        wq_sb = wq_f[:, csl].reshape(8, 128, 128).transpose(1, 0, 2).reshape(
            128, 1024)
        wk_sb = wk_f[:, csl].reshape(8, 128, 128).transpose(1, 0, 2).reshape(
            128, 1024)
        wv_sb = wv_f[:, csl].reshape(8, 128, 128).transpose(1, 0, 2).reshape(
            128, 1024)
        wpack = np.zeros((128, NWC), np.float32)
        wpack[:, OFF_WQ:OFF_WQ + 1024] = wq_sb
        wpack[:, OFF_WK:OFF_WK + 1024] = wk_sb
        wpack[:, OFF_WV:OFF_WV + 1024] = wv_sb
        wpack[:, OFF_W1:OFF_W1 + 1024] = Wout[csl, :]
        wpack[:, OFF_W2:OFF_W2 + 1024] = Wout[1024 + c * 128:
                                              1024 + (c + 1) * 128, :]
        wpack[:, OFF_WFQ:OFF_WFQ + 49] = wfq
        wpack[:, OFF_WFK:OFF_WFK + 49] = wfk
        wpack[:, OFF_WQF1:OFF_WQF1 + 16] = wqf1
        wpack[:, OFF_WKF1:OFF_WKF1 + 16] = wkf1
        wpack[:, OFF_MTRI2:OFF_MTRI2 + 256] = mtri2
        wpack[:, OFF_WINM:OFF_WINM + 512] = winm
        in_maps.append({
            "xT": xp,
            "wpack": wpack.astype(BF),
            "identf": np.eye(128, dtype=np.float32),
            "onesP": np.ones((1, P), np.float32).astype(BF),
        })
    return in_maps


def kernel(x, norm_w, Wq, Wk, Wv, Wqf, Wkf, Wout) -> np.ndarray:
    x = np.asarray(x, np.float32)
    in_maps = _host_prep(
        x, np.asarray(norm_w, np.float32), np.asarray(Wq, np.float32),
        np.asarray(Wk, np.float32), np.asarray(Wv, np.float32),
        np.asarray(Wqf, np.float32), np.asarray(Wkf, np.float32),
        np.asarray(Wout, np.float32))
    nc = _get_nc()
    res = run_bass_kernel_spmd(nc, in_maps, list(range(NCORES)))
    acc = np.zeros((P, D), np.float32)
    for c in range(NCORES):
        acc += res.results[c]["out"].astype(np.float32)
    return (x.reshape(P, D) + acc).reshape(B, T, D).astype(np.float32)
